# revision 6
# baseline (speedup 1.0000x reference)
"""GRU encoder (nn_Encoder_26087631356042) Bass/Trainium2 kernel.

Strategy: data-parallel over batch (B=128 -> 16 per core, 8 cores, no
collectives). Per core, a fused kernel: the input projection GEMM
(x @ W_ih.T) is computed 32 timesteps at a time inside the sequential
GRU time loop, entirely in feature-major "packed" layout
(feature f -> (block m = f//128, partition p = f%128)), so all gate
elementwise ops run with 128 active partitions and tiny free dims.

The recurrent matmul keeps W_hh.T stationary (bf16 hi+lo split) and
streams the hidden state (bf16 hi+lo split) as the moving operand,
accumulating exactly in fp32 PSUM; biases enter through a rank-1
"bias matmul" that also serves as the accumulation-group opener.
"""

import os
import numpy as np
import ml_dtypes
from contextlib import ExitStack

import concourse.bass as bass
import concourse.bacc as bacc
import concourse.tile as tile
import concourse.mybir as mybir
from concourse.bass_utils import run_bass_kernel_spmd

F32 = mybir.dt.float32
BF16 = mybir.dt.bfloat16
AF = mybir.ActivationFunctionType

B, T, X, H = 128, 2048, 128, 256
G = 3 * H          # 768 gate features
NBLK = G // 128    # 6 feature blocks
NCORES = 8
BL = B // NCORES   # 16 batch rows per core
CH = 64            # timesteps per For_i body
P = 128

bf16 = ml_dtypes.bfloat16


def _split_hi_lo(a32: np.ndarray):
    hi = a32.astype(bf16)
    lo = (a32 - hi.astype(np.float32)).astype(bf16)
    return hi, lo


def _build_program(t_steps: int, reps: int = 1, nogates: bool = False,
                   single: bool = False, nophase1: bool = False,
                   f32r_rhs: bool = False, allr: bool = False,
                   coltile: bool = False):
    """Emit the per-core program (same program on all cores; data differs).

    reps > 1 wraps the whole computation in an outer repeat loop (state
    carries over between reps — outputs are only timing-valid).
    nogates/single/nophase1 are timing-ablation variants."""
    nchunks = t_steps // CH
    nc = bacc.Bacc(
        "TRN2", target_bir_lowering=False, debug=False, num_devices=NCORES
    )

    # DRAM I/O
    d_xin_hi = nc.dram_tensor("xin_hi", [P, t_steps * BL], BF16, kind="ExternalInput")
    d_xin_lo = nc.dram_tensor("xin_lo", [P, t_steps * BL], BF16, kind="ExternalInput")
    d_whh_hi = nc.dram_tensor("whh_hi", [P, 2 * G], BF16, kind="ExternalInput")
    d_whh_lo = nc.dram_tensor("whh_lo", [P, 2 * G], BF16, kind="ExternalInput")
    d_wih_hi = nc.dram_tensor("wih_hi", [P, G], BF16, kind="ExternalInput")
    d_wih_lo = nc.dram_tensor("wih_lo", [P, G], BF16, kind="ExternalInput")
    d_biasmat = nc.dram_tensor("biasmat", [P, P], BF16, kind="ExternalInput")
    d_sel = nc.dram_tensor("sel", [P, NBLK * BL], BF16, kind="ExternalInput")
    d_bihn = nc.dram_tensor("bihn", [P, 2], F32, kind="ExternalInput")
    if allr:
        d_whh_f = nc.dram_tensor("whh_f", [P, 2 * G], F32, kind="ExternalInput")
        d_biasmat_f = nc.dram_tensor("biasmat_f", [P, P], F32, kind="ExternalInput")
        d_sel_f = nc.dram_tensor("sel_f", [P, NBLK * BL], F32, kind="ExternalInput")
    d_out = nc.dram_tensor("hout", [P, 2 * BL], F32, kind="ExternalOutput")

    with tile.TileContext(nc) as tc, ExitStack() as ctx:
        cpool = ctx.enter_context(tc.tile_pool(name="const", bufs=1))
        state = ctx.enter_context(tc.tile_pool(name="state", bufs=1))
        xpp = ctx.enter_context(tc.tile_pool(name="xp", bufs=1))
        xinp = ctx.enter_context(tc.tile_pool(name="xin", bufs=2))
        gsb = ctx.enter_context(tc.tile_pool(name="gates", bufs=2))
        php = ctx.enter_context(tc.tile_pool(name="php", bufs=2, space="PSUM"))
        phpn = ctx.enter_context(tc.tile_pool(name="phpn", bufs=2, space="PSUM"))
        pscr = ctx.enter_context(tc.tile_pool(name="pscr", bufs=2, space="PSUM"))
        px = ctx.enter_context(tc.tile_pool(name="px", bufs=2, space="PSUM"))

        # Constants -> SBUF
        whh_hi = cpool.tile([P, 2 * G], BF16, tag="whh_hi")
        whh_lo = cpool.tile([P, 2 * G], BF16, tag="whh_lo")
        wih_hi = cpool.tile([P, G], BF16, tag="wih_hi")
        wih_lo = cpool.tile([P, G], BF16, tag="wih_lo")
        biasmat = cpool.tile([P, P], BF16, tag="biasmat")
        sel = cpool.tile([P, NBLK * BL], BF16, tag="sel")
        bihn = cpool.tile([P, 2], F32, tag="bihn")
        loads = [
            (whh_hi, d_whh_hi), (whh_lo, d_whh_lo),
            (wih_hi, d_wih_hi), (wih_lo, d_wih_lo),
            (biasmat, d_biasmat), (sel, d_sel), (bihn, d_bihn),
        ]
        if allr:
            whh_f = cpool.tile([P, 2 * G], F32, tag="whh_f")
            biasmat_f = cpool.tile([P, P], F32, tag="biasmat_f")
            sel_f = cpool.tile([P, NBLK * BL], F32, tag="sel_f")
            loads += [(whh_f, d_whh_f), (biasmat_f, d_biasmat_f), (sel_f, d_sel_f)]
            whh_r = whh_f.bitcast(mybir.dt.float32r)
            biasmat_r = biasmat_f.bitcast(mybir.dt.float32r)
            sel_r = sel_f.bitcast(mybir.dt.float32r)
        for dst, src in loads:
            nc.sync.dma_start(dst[:], src.ap()[:])

        # Hidden state (feature-major packed): [128, 2 k-blocks, 16 batch]
        # Ping-pong pairs; CH is even so every body starts and ends on idx 0.
        hT = [state.tile([P, 2, BL], F32, name=f"hT{j}", tag=f"hT{j}") for j in range(2)]
        hTr = [t.bitcast(mybir.dt.float32r) for t in hT]
        hhi = [state.tile([P, 2, BL], BF16, name=f"hhi{j}", tag=f"hhi{j}") for j in range(2)]
        hlo = [state.tile([P, 2, BL], BF16, name=f"hlo{j}", tag=f"hlo{j}") for j in range(2)]
        for t_ in (hT[0], hhi[0], hlo[0]):
            nc.gpsimd.memset(t_[:], 0)

        # xp slab for one chunk: [128, 6 blocks, CH*BL cols] fp32
        xp = xpp.tile([P, NBLK, CH * BL], F32, tag="xp")
        if nophase1:
            nc.gpsimd.memset(xp[:], 0)

        def emit_time_loop():
          with tc.For_i(
            0, nchunks,
            hint_engines=(mybir.EngineType.PE, mybir.EngineType.DVE),
          ) as ci:
            # ---- Phase 1: xp = Wih @ x for CH steps (feature-major) ----
            xh = xinp.tile([P, CH * BL], BF16, tag="xh")
            xl = xinp.tile([P, CH * BL], BF16, tag="xl")
            nc.sync.dma_start(xh[:], d_xin_hi.ap()[:, bass.ts(ci, CH * BL)])
            nc.sync.dma_start(xl[:], d_xin_lo.ap()[:, bass.ts(ci, CH * BL)])
            for m in range(NBLK if not nophase1 else 0):
                for hf in range(CH * BL // 512):
                    pxm = px.tile([P, 512], F32, tag="pxm")
                    wsl = slice(128 * m, 128 * (m + 1))
                    xsl = slice(512 * hf, 512 * (hf + 1))
                    nc.tensor.matmul(pxm[:], wih_hi[:, wsl], xh[:, xsl],
                                     start=True, stop=False)
                    nc.tensor.matmul(pxm[:], wih_hi[:, wsl], xl[:, xsl],
                                     start=False, stop=False)
                    nc.tensor.matmul(pxm[:], wih_lo[:, wsl], xh[:, xsl],
                                     start=False, stop=True)
                    if m < 4:
                        nc.vector.tensor_copy(xp[:, m, xsl], pxm[:])
                    else:
                        # fold b_ih (n-gate part) in during evacuation
                        nc.scalar.activation(
                            xp[:, m, xsl], pxm[:], AF.Identity,
                            bias=bihn[:, m - 4: m - 3],
                        )

            # ---- Recurrence over CH steps ----
            for s in range(CH):
                cur, nxt = s % 2, (s + 1) % 2
                # split psum tiles: rz completes first so the sigmoid path
                # overlaps the n-block matmuls (deps are tile-granular)
                hprz = php.tile([P, 4, BL], F32, tag="hprz")
                hpn = phpn.tile([P, 2, BL], F32, tag="hpn")
                # bias matmuls open the accumulation groups (shared lhsT)
                bm = biasmat_r if allr else biasmat
                sl = sel_r if allr else sel
                nc.tensor.matmul(hprz.rearrange("p a b -> p (a b)"),
                                 bm[:], sl[:, 0:4 * BL],
                                 start=True, stop=False)
                nc.tensor.matmul(hpn.rearrange("p a b -> p (a b)"),
                                 bm[:], sl[:, 4 * BL:],
                                 start=True, stop=False)

                def emit_mms(ms, tgt, off):
                    for mi, m in enumerate(ms):
                        for k in range(2):
                            rh = hhi[cur][:, k, :]
                            rl = hlo[cur][:, k, :]
                            wsl = slice(G * k + 128 * m, G * k + 128 * (m + 1))
                            last = (k == 1 and mi == len(ms) - 1)
                            o = tgt[:, m - off, :]
                            if allr:
                                rf = hTr[cur][:, k, :]
                                nc.tensor.matmul(o, whh_r[:, wsl], rf,
                                                 start=False, stop=last)
                                continue
                            if f32r_rhs:
                                # exact h streamed as fp32r against bf16 weights
                                rf = hTr[cur][:, k, :]
                                nc.tensor.matmul(o, whh_hi[:, wsl], rf,
                                                 start=False, stop=False)
                                nc.tensor.matmul(o, whh_lo[:, wsl], rf,
                                                 start=False, stop=last)
                                continue
                            if single:
                                nc.tensor.matmul(o, whh_hi[:, wsl], rh,
                                                 start=False, stop=last)
                                continue
                            if coltile:
                                # [128,32] weight subtiles -> 4 col-groups of
                                # the PE array load + compute concurrently
                                base = G * k + 128 * m
                                for ti, (w, r) in enumerate(
                                    ((whh_hi, rh), (whh_hi, rl), (whh_lo, rh))
                                ):
                                    for q in range(4):
                                        qs = slice(base + 32 * q,
                                                   base + 32 * (q + 1))
                                        oq = o[32 * q: 32 * (q + 1), :]
                                        nc.tensor.matmul(
                                            oq, w[:, qs], r,
                                            start=False,
                                            stop=(last and ti == 2 and q == 3),
                                            tile_position=(0, 32 * q),
                                        )
                                continue
                            nc.tensor.matmul(o, whh_hi[:, wsl], rh,
                                             start=False, stop=False)
                            nc.tensor.matmul(o, whh_hi[:, wsl], rl,
                                             start=False, stop=False)
                            nc.tensor.matmul(o, whh_lo[:, wsl], rh,
                                             start=False, stop=last)

                emit_mms((0, 1, 2, 3), hprz, 0)
                emit_mms((4, 5), hpn, 4)

                xp_t = xp[:, :, bass.ts(s, BL)]          # [128, 6, 16]
                scr = pscr.tile([P, NBLK, BL], F32, tag="scr")
                rz = gsb.tile([P, 4, BL], F32, tag="rz")
                t1 = gsb.tile([P, 2, BL], F32, tag="t1")
                nsb = gsb.tile([P, 2, BL], F32, tag="nsb")
                zc = gsb.tile([P, 2, BL], F32, tag="zc")
                zh = gsb.tile([P, 2, BL], F32, tag="zh")
                t3 = gsb.tile([P, 2, BL], F32, tag="t3")

                if nogates:
                    # ablation: keep the serial dep chain, drop gate math
                    nc.vector.tensor_copy(hT[nxt][:], hpn[:, 0:2, :])
                    nc.vector.tensor_copy(hhi[nxt][:], hT[nxt][:])
                    nc.vector.tensor_sub(hlo[nxt][:], hT[nxt][:], hhi[nxt][:])
                    continue
                # r,z pre-activations then sigmoid (overlaps n-block MMs)
                nc.vector.tensor_add(scr[:, 0:4, :], xp_t[:, 0:4, :], hprz[:])
                nc.scalar.activation(rz[:], scr[:, 0:4, :], AF.Sigmoid)
                nc.scalar.activation(zc[:], rz[:, 2:4, :], AF.Copy,
                                     scale=-1.0, bias=1.0)
                # off-critical-path: z*h
                nc.vector.tensor_mul(zh[:], rz[:, 2:4, :], hT[cur][:])
                # n = tanh(xn + r*hn)   (b_ih_n already in xp, b_hh_n in hp)
                nc.vector.tensor_mul(t1[:], rz[:, 0:2, :], hpn[:])
                nc.vector.tensor_add(scr[:, 4:6, :], t1[:], xp_t[:, 4:6, :])
                nc.scalar.activation(nsb[:], scr[:, 4:6, :], AF.Tanh)
                # h' = (1-z)*n + z*h ; emit the bf16 hi part FIRST so the
                # next step's Whi@hhi matmuls can start one op earlier
                nc.vector.tensor_mul(t3[:], nsb[:], zc[:])
                nc.vector.tensor_add(hhi[nxt][:], t3[:], zh[:])
                nc.vector.tensor_add(hT[nxt][:], t3[:], zh[:])
                nc.vector.tensor_sub(hlo[nxt][:], hT[nxt][:], hhi[nxt][:])

        if reps > 1:
            with tc.For_i(0, reps, name="rep"):
                emit_time_loop()
        else:
            emit_time_loop()

        nc.sync.dma_start(d_out.ap()[:], hT[0].rearrange("p a b -> p (a b)"))

    nc.compile()
    return nc


def _build_v2(t_steps: int, reps: int = 1, p1pool: bool = True,
              n_interleave: int = 12):
    """v2: single-bf16 GRU step, xp folded into PSUM via identity matmul,
    STT-fused gate tail, phase-1 interleaved into the recurrence.

    Per-step serial chain: PE group (15 matmuls) -> ACT sigmoid(PSUM) ->
    DVE mul -> DVE add -> ACT tanh -> DVE STT -> DVE sub -> bf16 h'.
    """
    nchunks = t_steps // CH
    assert nchunks % 2 == 0
    halfiters = nchunks // 2
    nc = bacc.Bacc(
        "TRN2", target_bir_lowering=False, debug=False, num_devices=NCORES
    )

    # DRAM I/O (xin padded by 2 chunks so the in-loop prefetch stays in-bounds)
    d_xin = nc.dram_tensor("xin", [P, (t_steps + 2 * CH) * BL], BF16,
                           kind="ExternalInput")
    d_whh = nc.dram_tensor("whh", [P, 2 * G], BF16, kind="ExternalInput")
    d_wih = nc.dram_tensor("wih", [P, G], BF16, kind="ExternalInput")
    d_biasmat = nc.dram_tensor("biasmat", [P, P], BF16, kind="ExternalInput")
    d_sel = nc.dram_tensor("sel", [P, NBLK * BL], BF16, kind="ExternalInput")
    d_bihn = nc.dram_tensor("bihn", [P, 2], F32, kind="ExternalInput")
    d_ident = nc.dram_tensor("ident", [P, P], BF16, kind="ExternalInput")
    d_out = nc.dram_tensor("hout", [P, 2 * BL], BF16, kind="ExternalOutput")

    with tile.TileContext(nc) as tc, ExitStack() as ctx:
        cpool = ctx.enter_context(tc.tile_pool(name="const", bufs=1))
        state = ctx.enter_context(tc.tile_pool(name="state", bufs=1))
        gsb = ctx.enter_context(tc.tile_pool(name="gates", bufs=2))
        php = ctx.enter_context(tc.tile_pool(name="php", bufs=2, space="PSUM"))
        phpn = ctx.enter_context(tc.tile_pool(name="phpn", bufs=2, space="PSUM"))
        px = ctx.enter_context(tc.tile_pool(name="px", bufs=2, space="PSUM"))

        whh = cpool.tile([P, 2 * G], BF16, tag="whh")
        wih = cpool.tile([P, G], BF16, tag="wih")
        biasmat = cpool.tile([P, P], BF16, tag="biasmat")
        sel = cpool.tile([P, NBLK * BL], BF16, tag="sel")
        bihn = cpool.tile([P, 2], F32, tag="bihn")
        ident = cpool.tile([P, P], BF16, tag="ident")
        for dst, src in ((whh, d_whh), (wih, d_wih), (biasmat, d_biasmat),
                         (sel, d_sel), (bihn, d_bihn), (ident, d_ident)):
            nc.sync.dma_start(dst[:], src.ap()[:])

        # hidden state ping-pong, bf16 only
        h = [state.tile([P, 2, BL], BF16, name=f"h{j}", tag=f"h{j}")
             for j in range(2)]
        # xp slabs ping-pong (bf16), xin staging ping-pong
        xp2 = [state.tile([P, NBLK, CH * BL], BF16, name=f"xp{j}", tag=f"xp{j}")
               for j in range(2)]
        xin2 = [state.tile([P, CH * BL], BF16, name=f"xin{j}", tag=f"xin{j}")
                for j in range(2)]

        def emit_p1_item(item, xin_t, xp_t):
            """Phase-1 item `item` in 0..11: matmul (m, hf) + evacuation.
            GPSIMD can't read PSUM, so evacuate on DVE (plain copies) and
            ACT (the two n-blocks that fold in b_ih_n)."""
            m, hf = item // 2, item % 2
            pxm = px.tile([P, 512], F32, tag="pxm")
            wsl = slice(128 * m, 128 * (m + 1))
            xsl = slice(512 * hf, 512 * (hf + 1))
            nc.tensor.matmul(pxm[:], wih[:, wsl], xin_t[:, xsl],
                             start=True, stop=True)
            if m < 4:
                nc.vector.tensor_copy(xp_t[:, m, xsl], pxm[:])
            else:
                nc.scalar.activation(xp_t[:, m, xsl], pxm[:], AF.Identity,
                                     bias=bihn[:, m - 4: m - 3])

        def emit_step(s, xp_t):
            cur, nxt = s % 2, (s + 1) % 2
            hprz = php.tile([P, 4, BL], F32, tag="hprz")
            hpn = phpn.tile([P, 2, BL], F32, tag="hpn")
            ssl = slice(BL * s, BL * (s + 1))
            # rz accumulation group: bias opener, 8 Whh mms, xp fold closer
            nc.tensor.matmul(hprz.rearrange("p a b -> p (a b)"),
                             biasmat[:], sel[:, 0:4 * BL],
                             start=True, stop=False)
            for k in range(2):
                for m in range(4):
                    wsl = slice(G * k + 128 * m, G * k + 128 * (m + 1))
                    nc.tensor.matmul(hprz[:, m, :], whh[:, wsl], h[cur][:, k, :],
                                     start=False, stop=False)
            nc.tensor.matmul(hprz[:], ident[:], xp_t[:, 0:4, ssl],
                             start=False, stop=True)
            # n accumulation group
            nc.tensor.matmul(hpn.rearrange("p a b -> p (a b)"),
                             biasmat[:], sel[:, 4 * BL:6 * BL],
                             start=True, stop=False)
            for k in range(2):
                for mi, m in enumerate((4, 5)):
                    wsl = slice(G * k + 128 * m, G * k + 128 * (m + 1))
                    nc.tensor.matmul(hpn[:, mi, :], whh[:, wsl], h[cur][:, k, :],
                                     start=False, stop=(k == 1 and mi == 1))

            rz = gsb.tile([P, 4, BL], F32, tag="rz")
            t1 = gsb.tile([P, 2, BL], F32, tag="t1")
            sn = gsb.tile([P, 2, BL], F32, tag="sn")
            zh = gsb.tile([P, 2, BL], F32, tag="zh")
            nt = gsb.tile([P, 2, BL], F32, tag="nt")
            t3m = gsb.tile([P, 2, BL], F32, tag="t3m")
            nc.scalar.activation(rz[:], hprz[:], AF.Sigmoid)
            nc.vector.tensor_mul(t1[:], rz[:, 0:2, :], hpn[:])
            nc.vector.tensor_add(sn[:], t1[:], xp_t[:, 4:6, ssl])
            # off-critical-path z*h (runs on DVE while ACT does tanh)
            nc.vector.tensor_mul(zh[:], rz[:, 2:4, :], h[cur][:])
            nc.scalar.activation(nt[:], sn[:], AF.Tanh)
            # h' = z*h - (z-1)*n = z*h + (1-z)*n
            nc.vector.scalar_tensor_tensor(
                t3m[:], rz[:, 2:4, :], 1.0, nt[:],
                op0=mybir.AluOpType.subtract, op1=mybir.AluOpType.mult)
            nc.vector.tensor_sub(h[nxt][:], zh[:], t3m[:])

        def emit_half(ci, parity):
            """Recurrence for chunk (2*ci+parity) reading xp2[parity];
            interleaved phase-1 for the next chunk into xp2[1-parity];
            prefetch DMA for chunk (2*ci+parity+2) into xin2[parity]."""
            cols = CH * BL
            nc.sync.dma_start(
                xin2[parity][:],
                d_xin.ap()[:, bass.ds(ci * (2 * cols) + (parity + 2) * cols, cols)])
            for s in range(CH):
                emit_step(s, xp2[parity])
                if s < n_interleave:
                    emit_p1_item(s, xin2[1 - parity], xp2[1 - parity])
                elif n_interleave == 0 and s == 0:
                    for it in range(12):
                        emit_p1_item(it, xin2[1 - parity], xp2[1 - parity])

        def emit_all():
            for t_ in (h[0],):
                nc.gpsimd.memset(t_[:], 0)
            # prologue: xin2[j] holds chunks of parity j throughout.
            nc.sync.dma_start(xin2[0][:], d_xin.ap()[:, 0:CH * BL])
            for it in range(12):
                emit_p1_item(it, xin2[0], xp2[0])
            # stage chunk 1 (consumed by half parity=0's interleaved phase-1)
            nc.sync.dma_start(xin2[1][:], d_xin.ap()[:, CH * BL:2 * CH * BL])
            with tc.For_i(
                0, halfiters,
                hint_engines=(mybir.EngineType.PE, mybir.EngineType.DVE),
            ) as ci:
                emit_half(ci, 0)
                emit_half(ci, 1)

        if reps > 1:
            with tc.For_i(0, reps, name="rep"):
                emit_all()
        else:
            emit_all()

        nc.sync.dma_start(d_out.ap()[:], h[0].rearrange("p a b -> p (a b)"))

    nc.compile()
    return nc


def _pack_v2(input, W_ih, W_hh, b_ih, b_hh, t_steps: int):
    input = np.asarray(input, np.float32)
    W_ih = np.asarray(W_ih, np.float32)
    W_hh = np.asarray(W_hh, np.float32)
    b_ih = np.asarray(b_ih, np.float32)
    b_hh = np.asarray(b_hh, np.float32)

    whhT = np.ascontiguousarray(W_hh.T)              # [H, G]
    whh = whhT.reshape(2, P, G).transpose(1, 0, 2).reshape(P, 2 * G)
    whh = np.ascontiguousarray(whh).astype(bf16)
    wih = np.ascontiguousarray(W_ih.T).astype(bf16)  # [128, 768]

    bias_full = b_hh.copy()
    bias_full[: 2 * H] += b_ih[: 2 * H]
    bmat32 = np.zeros((P, P), np.float32)
    bvec = bias_full.reshape(NBLK, P)
    bhi = bvec.astype(bf16).astype(np.float32)
    bmat32[0:NBLK, :] = bhi
    bmat32[NBLK: 2 * NBLK, :] = bvec - bhi
    biasmat = bmat32.astype(bf16)
    selmat = np.zeros((P, NBLK * BL), np.float32)
    for m in range(NBLK):
        selmat[m, BL * m: BL * (m + 1)] = 1.0
        selmat[m + NBLK, BL * m: BL * (m + 1)] = 1.0
    sel = selmat.astype(bf16)
    bihn = np.ascontiguousarray(b_ih[2 * H:].reshape(2, P).T)  # [128, 2]
    ident = np.eye(P, dtype=np.float32).astype(bf16)

    shared = dict(whh=whh, wih=wih, biasmat=biasmat, sel=sel, bihn=bihn,
                  ident=ident)
    pad = 2 * CH * BL
    in_maps = []
    for c in range(NCORES):
        xs = input[c * BL: (c + 1) * BL, :t_steps, :]     # [16, t, 128]
        xt = np.ascontiguousarray(xs.transpose(2, 1, 0))  # [128, t, 16]
        xt = xt.reshape(P, t_steps * BL).astype(bf16)
        xin = np.zeros((P, t_steps * BL + pad), bf16)
        xin[:, :t_steps * BL] = xt
        m = dict(shared)
        m["xin"] = xin
        in_maps.append(m)
    return in_maps


def _unpack_v2(results):
    out = np.empty((B, H), np.float32)
    for c in range(NCORES):
        o = results[c]["hout"].astype(np.float32).reshape(P, 2, BL)
        out[c * BL: (c + 1) * BL, :] = o.transpose(2, 1, 0).reshape(BL, H)
    return out


_PROGRAM_CACHE: dict = {}


def _get_program(t_steps: int, reps: int = 1, ver: int = 2):
    key = (t_steps, reps, ver)
    if key not in _PROGRAM_CACHE:
        builder = _build_v2 if ver == 2 else _build_program
        _PROGRAM_CACHE[key] = builder(t_steps, reps)
    return _PROGRAM_CACHE[key]


def _pack_inputs(input, W_ih, W_hh, b_ih, b_hh, t_steps: int):
    """Host-side packing. Returns per-core in_maps."""
    input = np.asarray(input, np.float32)
    W_ih = np.asarray(W_ih, np.float32)
    W_hh = np.asarray(W_hh, np.float32)
    b_ih = np.asarray(b_ih, np.float32)
    b_hh = np.asarray(b_hh, np.float32)

    # weights, feature-major packed (shared by all cores)
    whhT = np.ascontiguousarray(W_hh.T)              # [H, G]
    whh = whhT.reshape(2, P, G).transpose(1, 0, 2).reshape(P, 2 * G)
    whh_hi, whh_lo = _split_hi_lo(np.ascontiguousarray(whh))
    wihT = np.ascontiguousarray(W_ih.T)              # [X, G] = [128, 768]
    wih_hi, wih_lo = _split_hi_lo(wihT)

    # bias matrix: rows 0..5 hi parts, rows 6..11 lo parts; selector picks both
    bias_full = b_hh.copy()
    bias_full[: 2 * H] += b_ih[: 2 * H]              # r,z: b_ih + b_hh; n: b_hh
    bmat32 = np.zeros((P, P), np.float32)
    bvec = bias_full.reshape(NBLK, P)
    bhi = bvec.astype(bf16).astype(np.float32)
    blo = bvec - bhi
    bmat32[0:NBLK, :] = bhi
    bmat32[NBLK: 2 * NBLK, :] = blo
    biasmat = bmat32.astype(bf16)
    selmat = np.zeros((P, NBLK * BL), np.float32)
    for m in range(NBLK):
        selmat[m, BL * m: BL * (m + 1)] = 1.0
        selmat[m + NBLK, BL * m: BL * (m + 1)] = 1.0
    sel = selmat.astype(bf16)
    bihn = np.ascontiguousarray(b_ih[2 * H:].reshape(2, P).T)  # [128, 2]

    shared = dict(
        whh_hi=whh_hi, whh_lo=whh_lo, wih_hi=wih_hi, wih_lo=wih_lo,
        biasmat=biasmat, sel=sel, bihn=bihn,
        whh_f=np.ascontiguousarray(whh), biasmat_f=bmat32, sel_f=selmat,
    )
    in_maps = []
    for c in range(NCORES):
        xs = input[c * BL: (c + 1) * BL, :t_steps, :]     # [16, t, 128]
        xt = np.ascontiguousarray(xs.transpose(2, 1, 0))  # [128, t, 16]
        xt = xt.reshape(P, t_steps * BL)
        xh, xl = _split_hi_lo(xt)
        m = dict(shared)
        m["xin_hi"] = xh
        m["xin_lo"] = xl
        in_maps.append(m)
    return in_maps


def _unpack_output(results):
    out = np.empty((B, H), np.float32)
    for c in range(NCORES):
        o = results[c]["hout"].reshape(P, 2, BL)           # [p, k, b]
        out[c * BL: (c + 1) * BL, :] = o.transpose(2, 1, 0).reshape(BL, H)
    return out


VER = 2


def run(input, W_ih, W_hh, b_ih, b_hh, t_steps: int = T, trace: bool = False,
        ver: int = None):
    ver = VER if ver is None else ver
    nc = _get_program(t_steps, ver=ver)
    pack = _pack_v2 if ver == 2 else _pack_inputs
    unpack = _unpack_v2 if ver == 2 else _unpack_output
    in_maps = pack(input, W_ih, W_hh, b_ih, b_hh, t_steps)
    res = run_bass_kernel_spmd(
        nc, in_maps, core_ids=list(range(NCORES)), trace=trace
    )
    return unpack(res.results), res


def kernel(input, W_ih, W_hh, b_ih, b_hh):
    out, _ = run(input, W_ih, W_hh, b_ih, b_hh)
    return out


def bench(input, W_ih, W_hh, b_ih, b_hh, reps_hi: int = 5, iters: int = 3,
          ver: int = None):
    """Estimate on-device time: wall(R=reps_hi) - wall(R=1) over cached
    executables, divided by (reps_hi - 1). Returns ns."""
    import time as _time

    ver = VER if ver is None else ver
    pack = _pack_v2 if ver == 2 else _pack_inputs
    in_maps = pack(input, W_ih, W_hh, b_ih, b_hh, T)
    nc1 = _get_program(T, 1, ver=ver)
    ncR = _get_program(T, reps_hi, ver=ver)

    def timed(nc):
        best = float("inf")
        for _ in range(iters):
            t0 = _time.perf_counter()
            run_bass_kernel_spmd(nc, in_maps, core_ids=list(range(NCORES)))
            best = min(best, _time.perf_counter() - t0)
        return best

    # warm both executables (compile cache)
    run_bass_kernel_spmd(nc1, in_maps, core_ids=list(range(NCORES)))
    run_bass_kernel_spmd(ncR, in_maps, core_ids=list(range(NCORES)))
    t1 = timed(nc1)
    tR = timed(ncR)
    ns = (tR - t1) / (reps_hi - 1) * 1e9
    print(f"wall R=1: {t1*1e3:.1f} ms   wall R={reps_hi}: {tR*1e3:.1f} ms")
    return ns



# revision 21
# speedup vs baseline: 13.1619x; 13.1619x over previous
"""GRU encoder (nn_Encoder_26087631356042) Bass/Trainium2 kernel.

Strategy: data-parallel over batch (B=128 -> 16 per core, 8 cores, no
collectives). Per core, a fused kernel: the input projection GEMM
(x @ W_ih.T) is computed 32 timesteps at a time inside the sequential
GRU time loop, entirely in feature-major "packed" layout
(feature f -> (block m = f//128, partition p = f%128)), so all gate
elementwise ops run with 128 active partitions and tiny free dims.

The recurrent matmul keeps W_hh.T stationary (bf16 hi+lo split) and
streams the hidden state (bf16 hi+lo split) as the moving operand,
accumulating exactly in fp32 PSUM; biases enter through a rank-1
"bias matmul" that also serves as the accumulation-group opener.
"""

import os
import numpy as np
import ml_dtypes
from contextlib import ExitStack

import concourse.bass as bass
import concourse.bacc as bacc
import concourse.tile as tile
import concourse.mybir as mybir
from concourse.bass_utils import run_bass_kernel_spmd

F32 = mybir.dt.float32
BF16 = mybir.dt.bfloat16
AF = mybir.ActivationFunctionType

B, T, X, H = 128, 2048, 128, 256
G = 3 * H          # 768 gate features
NBLK = G // 128    # 6 feature blocks
NCORES = 8
BL = B // NCORES   # 16 batch rows per core
CH = 64            # timesteps per For_i body
P = 128

bf16 = ml_dtypes.bfloat16


def _split_hi_lo(a32: np.ndarray):
    hi = a32.astype(bf16)
    lo = (a32 - hi.astype(np.float32)).astype(bf16)
    return hi, lo


def _build_program(t_steps: int, reps: int = 1, nogates: bool = False,
                   single: bool = False, nophase1: bool = False,
                   f32r_rhs: bool = False, allr: bool = False,
                   coltile: bool = False):
    """Emit the per-core program (same program on all cores; data differs).

    reps > 1 wraps the whole computation in an outer repeat loop (state
    carries over between reps — outputs are only timing-valid).
    nogates/single/nophase1 are timing-ablation variants."""
    nchunks = t_steps // CH
    nc = bacc.Bacc(
        "TRN2", target_bir_lowering=False, debug=False, num_devices=NCORES
    )

    # DRAM I/O
    d_xin_hi = nc.dram_tensor("xin_hi", [P, t_steps * BL], BF16, kind="ExternalInput")
    d_xin_lo = nc.dram_tensor("xin_lo", [P, t_steps * BL], BF16, kind="ExternalInput")
    d_whh_hi = nc.dram_tensor("whh_hi", [P, 2 * G], BF16, kind="ExternalInput")
    d_whh_lo = nc.dram_tensor("whh_lo", [P, 2 * G], BF16, kind="ExternalInput")
    d_wih_hi = nc.dram_tensor("wih_hi", [P, G], BF16, kind="ExternalInput")
    d_wih_lo = nc.dram_tensor("wih_lo", [P, G], BF16, kind="ExternalInput")
    d_biasmat = nc.dram_tensor("biasmat", [P, P], BF16, kind="ExternalInput")
    d_sel = nc.dram_tensor("sel", [P, NBLK * BL], BF16, kind="ExternalInput")
    d_bihn = nc.dram_tensor("bihn", [P, 2], F32, kind="ExternalInput")
    if allr:
        d_whh_f = nc.dram_tensor("whh_f", [P, 2 * G], F32, kind="ExternalInput")
        d_biasmat_f = nc.dram_tensor("biasmat_f", [P, P], F32, kind="ExternalInput")
        d_sel_f = nc.dram_tensor("sel_f", [P, NBLK * BL], F32, kind="ExternalInput")
    d_out = nc.dram_tensor("hout", [P, 2 * BL], F32, kind="ExternalOutput")

    with tile.TileContext(nc) as tc, ExitStack() as ctx:
        cpool = ctx.enter_context(tc.tile_pool(name="const", bufs=1))
        state = ctx.enter_context(tc.tile_pool(name="state", bufs=1))
        xpp = ctx.enter_context(tc.tile_pool(name="xp", bufs=1))
        xinp = ctx.enter_context(tc.tile_pool(name="xin", bufs=2))
        gsb = ctx.enter_context(tc.tile_pool(name="gates", bufs=2))
        php = ctx.enter_context(tc.tile_pool(name="php", bufs=2, space="PSUM"))
        phpn = ctx.enter_context(tc.tile_pool(name="phpn", bufs=2, space="PSUM"))
        pscr = ctx.enter_context(tc.tile_pool(name="pscr", bufs=2, space="PSUM"))
        px = ctx.enter_context(tc.tile_pool(name="px", bufs=2, space="PSUM"))

        # Constants -> SBUF
        whh_hi = cpool.tile([P, 2 * G], BF16, tag="whh_hi")
        whh_lo = cpool.tile([P, 2 * G], BF16, tag="whh_lo")
        wih_hi = cpool.tile([P, G], BF16, tag="wih_hi")
        wih_lo = cpool.tile([P, G], BF16, tag="wih_lo")
        biasmat = cpool.tile([P, P], BF16, tag="biasmat")
        sel = cpool.tile([P, NBLK * BL], BF16, tag="sel")
        bihn = cpool.tile([P, 2], F32, tag="bihn")
        loads = [
            (whh_hi, d_whh_hi), (whh_lo, d_whh_lo),
            (wih_hi, d_wih_hi), (wih_lo, d_wih_lo),
            (biasmat, d_biasmat), (sel, d_sel), (bihn, d_bihn),
        ]
        if allr:
            whh_f = cpool.tile([P, 2 * G], F32, tag="whh_f")
            biasmat_f = cpool.tile([P, P], F32, tag="biasmat_f")
            sel_f = cpool.tile([P, NBLK * BL], F32, tag="sel_f")
            loads += [(whh_f, d_whh_f), (biasmat_f, d_biasmat_f), (sel_f, d_sel_f)]
            whh_r = whh_f.bitcast(mybir.dt.float32r)
            biasmat_r = biasmat_f.bitcast(mybir.dt.float32r)
            sel_r = sel_f.bitcast(mybir.dt.float32r)
        for dst, src in loads:
            nc.sync.dma_start(dst[:], src.ap()[:])

        # Hidden state (feature-major packed): [128, 2 k-blocks, 16 batch]
        # Ping-pong pairs; CH is even so every body starts and ends on idx 0.
        hT = [state.tile([P, 2, BL], F32, name=f"hT{j}", tag=f"hT{j}") for j in range(2)]
        hTr = [t.bitcast(mybir.dt.float32r) for t in hT]
        hhi = [state.tile([P, 2, BL], BF16, name=f"hhi{j}", tag=f"hhi{j}") for j in range(2)]
        hlo = [state.tile([P, 2, BL], BF16, name=f"hlo{j}", tag=f"hlo{j}") for j in range(2)]
        for t_ in (hT[0], hhi[0], hlo[0]):
            nc.gpsimd.memset(t_[:], 0)

        # xp slab for one chunk: [128, 6 blocks, CH*BL cols] fp32
        xp = xpp.tile([P, NBLK, CH * BL], F32, tag="xp")
        if nophase1:
            nc.gpsimd.memset(xp[:], 0)

        def emit_time_loop():
          with tc.For_i(
            0, nchunks,
            hint_engines=(mybir.EngineType.PE, mybir.EngineType.DVE),
          ) as ci:
            # ---- Phase 1: xp = Wih @ x for CH steps (feature-major) ----
            xh = xinp.tile([P, CH * BL], BF16, tag="xh")
            xl = xinp.tile([P, CH * BL], BF16, tag="xl")
            nc.sync.dma_start(xh[:], d_xin_hi.ap()[:, bass.ts(ci, CH * BL)])
            nc.sync.dma_start(xl[:], d_xin_lo.ap()[:, bass.ts(ci, CH * BL)])
            for m in range(NBLK if not nophase1 else 0):
                for hf in range(CH * BL // 512):
                    pxm = px.tile([P, 512], F32, tag="pxm")
                    wsl = slice(128 * m, 128 * (m + 1))
                    xsl = slice(512 * hf, 512 * (hf + 1))
                    nc.tensor.matmul(pxm[:], wih_hi[:, wsl], xh[:, xsl],
                                     start=True, stop=False)
                    nc.tensor.matmul(pxm[:], wih_hi[:, wsl], xl[:, xsl],
                                     start=False, stop=False)
                    nc.tensor.matmul(pxm[:], wih_lo[:, wsl], xh[:, xsl],
                                     start=False, stop=True)
                    if m < 4:
                        nc.vector.tensor_copy(xp[:, m, xsl], pxm[:])
                    else:
                        # fold b_ih (n-gate part) in during evacuation
                        nc.scalar.activation(
                            xp[:, m, xsl], pxm[:], AF.Identity,
                            bias=bihn[:, m - 4: m - 3],
                        )

            # ---- Recurrence over CH steps ----
            for s in range(CH):
                cur, nxt = s % 2, (s + 1) % 2
                # split psum tiles: rz completes first so the sigmoid path
                # overlaps the n-block matmuls (deps are tile-granular)
                hprz = php.tile([P, 4, BL], F32, tag="hprz")
                hpn = phpn.tile([P, 2, BL], F32, tag="hpn")
                # bias matmuls open the accumulation groups (shared lhsT)
                bm = biasmat_r if allr else biasmat
                sl = sel_r if allr else sel
                nc.tensor.matmul(hprz.rearrange("p a b -> p (a b)"),
                                 bm[:], sl[:, 0:4 * BL],
                                 start=True, stop=False)
                nc.tensor.matmul(hpn.rearrange("p a b -> p (a b)"),
                                 bm[:], sl[:, 4 * BL:],
                                 start=True, stop=False)

                def emit_mms(ms, tgt, off):
                    for mi, m in enumerate(ms):
                        for k in range(2):
                            rh = hhi[cur][:, k, :]
                            rl = hlo[cur][:, k, :]
                            wsl = slice(G * k + 128 * m, G * k + 128 * (m + 1))
                            last = (k == 1 and mi == len(ms) - 1)
                            o = tgt[:, m - off, :]
                            if allr:
                                rf = hTr[cur][:, k, :]
                                nc.tensor.matmul(o, whh_r[:, wsl], rf,
                                                 start=False, stop=last)
                                continue
                            if f32r_rhs:
                                # exact h streamed as fp32r against bf16 weights
                                rf = hTr[cur][:, k, :]
                                nc.tensor.matmul(o, whh_hi[:, wsl], rf,
                                                 start=False, stop=False)
                                nc.tensor.matmul(o, whh_lo[:, wsl], rf,
                                                 start=False, stop=last)
                                continue
                            if single:
                                nc.tensor.matmul(o, whh_hi[:, wsl], rh,
                                                 start=False, stop=last)
                                continue
                            if coltile:
                                # [128,32] weight subtiles -> 4 col-groups of
                                # the PE array load + compute concurrently
                                base = G * k + 128 * m
                                for ti, (w, r) in enumerate(
                                    ((whh_hi, rh), (whh_hi, rl), (whh_lo, rh))
                                ):
                                    for q in range(4):
                                        qs = slice(base + 32 * q,
                                                   base + 32 * (q + 1))
                                        oq = o[32 * q: 32 * (q + 1), :]
                                        nc.tensor.matmul(
                                            oq, w[:, qs], r,
                                            start=False,
                                            stop=(last and ti == 2 and q == 3),
                                            tile_position=(0, 32 * q),
                                        )
                                continue
                            nc.tensor.matmul(o, whh_hi[:, wsl], rh,
                                             start=False, stop=False)
                            nc.tensor.matmul(o, whh_hi[:, wsl], rl,
                                             start=False, stop=False)
                            nc.tensor.matmul(o, whh_lo[:, wsl], rh,
                                             start=False, stop=last)

                emit_mms((0, 1, 2, 3), hprz, 0)
                emit_mms((4, 5), hpn, 4)

                xp_t = xp[:, :, bass.ts(s, BL)]          # [128, 6, 16]
                scr = pscr.tile([P, NBLK, BL], F32, tag="scr")
                rz = gsb.tile([P, 4, BL], F32, tag="rz")
                t1 = gsb.tile([P, 2, BL], F32, tag="t1")
                nsb = gsb.tile([P, 2, BL], F32, tag="nsb")
                zc = gsb.tile([P, 2, BL], F32, tag="zc")
                zh = gsb.tile([P, 2, BL], F32, tag="zh")
                t3 = gsb.tile([P, 2, BL], F32, tag="t3")

                if nogates:
                    # ablation: keep the serial dep chain, drop gate math
                    nc.vector.tensor_copy(hT[nxt][:], hpn[:, 0:2, :])
                    nc.vector.tensor_copy(hhi[nxt][:], hT[nxt][:])
                    nc.vector.tensor_sub(hlo[nxt][:], hT[nxt][:], hhi[nxt][:])
                    continue
                # r,z pre-activations then sigmoid (overlaps n-block MMs)
                nc.vector.tensor_add(scr[:, 0:4, :], xp_t[:, 0:4, :], hprz[:])
                nc.scalar.activation(rz[:], scr[:, 0:4, :], AF.Sigmoid)
                nc.scalar.activation(zc[:], rz[:, 2:4, :], AF.Copy,
                                     scale=-1.0, bias=1.0)
                # off-critical-path: z*h
                nc.vector.tensor_mul(zh[:], rz[:, 2:4, :], hT[cur][:])
                # n = tanh(xn + r*hn)   (b_ih_n already in xp, b_hh_n in hp)
                nc.vector.tensor_mul(t1[:], rz[:, 0:2, :], hpn[:])
                nc.vector.tensor_add(scr[:, 4:6, :], t1[:], xp_t[:, 4:6, :])
                nc.scalar.activation(nsb[:], scr[:, 4:6, :], AF.Tanh)
                # h' = (1-z)*n + z*h ; emit the bf16 hi part FIRST so the
                # next step's Whi@hhi matmuls can start one op earlier
                nc.vector.tensor_mul(t3[:], nsb[:], zc[:])
                nc.vector.tensor_add(hhi[nxt][:], t3[:], zh[:])
                nc.vector.tensor_add(hT[nxt][:], t3[:], zh[:])
                nc.vector.tensor_sub(hlo[nxt][:], hT[nxt][:], hhi[nxt][:])

        if reps > 1:
            with tc.For_i(0, reps, name="rep"):
                emit_time_loop()
        else:
            emit_time_loop()

        nc.sync.dma_start(d_out.ap()[:], hT[0].rearrange("p a b -> p (a b)"))

    nc.compile()
    return nc


def _build_v2(t_steps: int, reps: int = 1, p1pool: bool = True,
              n_interleave: int = 12, act_evac: bool = True,
              fold: bool = True, stt: bool = True, nodma: bool = False):
    """v2: single-bf16 GRU step, xp folded into PSUM via identity matmul,
    STT-fused gate tail, phase-1 interleaved into the recurrence.

    Per-step serial chain: PE group (15 matmuls) -> ACT sigmoid(PSUM) ->
    DVE mul -> DVE add -> ACT tanh -> DVE STT -> DVE sub -> bf16 h'.
    """
    nchunks = t_steps // CH
    assert nchunks % 2 == 0
    halfiters = nchunks // 2
    nc = bacc.Bacc(
        "TRN2", target_bir_lowering=False, debug=False, num_devices=NCORES
    )

    # DRAM I/O (xin padded by 2 chunks so the in-loop prefetch stays in-bounds)
    d_xin = nc.dram_tensor("xin", [P, (t_steps + 2 * CH) * BL], BF16,
                           kind="ExternalInput")
    d_whh = nc.dram_tensor("whh", [P, 2 * G], BF16, kind="ExternalInput")
    d_wih = nc.dram_tensor("wih", [P, G], BF16, kind="ExternalInput")
    d_biasmat = nc.dram_tensor("biasmat", [P, P], BF16, kind="ExternalInput")
    d_sel = nc.dram_tensor("sel", [P, NBLK * BL], BF16, kind="ExternalInput")
    d_bihn = nc.dram_tensor("bihn", [P, 2], F32, kind="ExternalInput")
    d_ident = nc.dram_tensor("ident", [P, P], BF16, kind="ExternalInput")
    d_out = nc.dram_tensor("hout", [P, 2 * BL], BF16, kind="ExternalOutput")

    with tile.TileContext(nc) as tc, ExitStack() as ctx:
        cpool = ctx.enter_context(tc.tile_pool(name="const", bufs=1))
        state = ctx.enter_context(tc.tile_pool(name="state", bufs=1))
        gsb = ctx.enter_context(tc.tile_pool(name="gates", bufs=2))
        php = ctx.enter_context(tc.tile_pool(name="php", bufs=2, space="PSUM"))
        phpn = ctx.enter_context(tc.tile_pool(name="phpn", bufs=2, space="PSUM"))
        px = ctx.enter_context(tc.tile_pool(name="px", bufs=2, space="PSUM"))

        whh = cpool.tile([P, 2 * G], BF16, tag="whh")
        wih = cpool.tile([P, G], BF16, tag="wih")
        biasmat = cpool.tile([P, P], BF16, tag="biasmat")
        sel = cpool.tile([P, NBLK * BL], BF16, tag="sel")
        bihn = cpool.tile([P, 2], F32, tag="bihn")
        ident = cpool.tile([P, P], BF16, tag="ident")
        for dst, src in ((whh, d_whh), (wih, d_wih), (biasmat, d_biasmat),
                         (sel, d_sel), (bihn, d_bihn), (ident, d_ident)):
            nc.sync.dma_start(dst[:], src.ap()[:])

        # hidden state ping-pong, bf16 only
        h = [state.tile([P, 2, BL], BF16, name=f"h{j}", tag=f"h{j}")
             for j in range(2)]
        # xp slabs ping-pong (bf16), xin staging ping-pong
        xp2 = [state.tile([P, NBLK, CH * BL], BF16, name=f"xp{j}", tag=f"xp{j}")
               for j in range(2)]
        xin2 = [state.tile([P, CH * BL], BF16, name=f"xin{j}", tag=f"xin{j}")
                for j in range(2)]

        def emit_p1_item(item, xin_t, xp_t):
            """Phase-1 item `item` in 0..11: matmul (m, hf) + evacuation.
            GPSIMD can't read PSUM, so evacuate on DVE (plain copies) and
            ACT (the two n-blocks that fold in b_ih_n)."""
            m, hf = item // 2, item % 2
            pxm = px.tile([P, 512], F32, tag="pxm")
            wsl = slice(128 * m, 128 * (m + 1))
            xsl = slice(512 * hf, 512 * (hf + 1))
            nc.tensor.matmul(pxm[:], wih[:, wsl], xin_t[:, xsl],
                             start=True, stop=True)
            if m < 4:
                nc.vector.tensor_copy(xp_t[:, m, xsl], pxm[:])
            elif act_evac:
                nc.scalar.activation(xp_t[:, m, xsl], pxm[:], AF.Identity,
                                     bias=bihn[:, m - 4: m - 3])
            else:
                nc.vector.tensor_scalar_add(xp_t[:, m, xsl], pxm[:],
                                            bihn[:, m - 4: m - 3])

        def emit_step(s, xp_t):
            cur, nxt = s % 2, (s + 1) % 2
            hprz = php.tile([P, 4, BL], F32, tag="hprz")
            hpn = phpn.tile([P, 2, BL], F32, tag="hpn")
            ssl = slice(BL * s, BL * (s + 1))
            # rz accumulation group: bias opener, 8 Whh mms, xp fold closer
            nc.tensor.matmul(hprz.rearrange("p a b -> p (a b)"),
                             biasmat[:], sel[:, 0:4 * BL],
                             start=True, stop=False)
            for k in range(2):
                for m in range(4):
                    wsl = slice(G * k + 128 * m, G * k + 128 * (m + 1))
                    last = not fold and (k == 1 and m == 3)
                    nc.tensor.matmul(hprz[:, m, :], whh[:, wsl], h[cur][:, k, :],
                                     start=False, stop=last)
            if fold:
                nc.tensor.matmul(hprz[:], ident[:], xp_t[:, 0:4, ssl],
                                 start=False, stop=True)
            # n accumulation group
            nc.tensor.matmul(hpn.rearrange("p a b -> p (a b)"),
                             biasmat[:], sel[:, 4 * BL:6 * BL],
                             start=True, stop=False)
            for k in range(2):
                for mi, m in enumerate((4, 5)):
                    wsl = slice(G * k + 128 * m, G * k + 128 * (m + 1))
                    nc.tensor.matmul(hpn[:, mi, :], whh[:, wsl], h[cur][:, k, :],
                                     start=False, stop=(k == 1 and mi == 1))

            rz = gsb.tile([P, 4, BL], F32, tag="rz")
            t1 = gsb.tile([P, 2, BL], F32, tag="t1")
            sn = gsb.tile([P, 2, BL], F32, tag="sn")
            zh = gsb.tile([P, 2, BL], F32, tag="zh")
            nt = gsb.tile([P, 2, BL], F32, tag="nt")
            t3m = gsb.tile([P, 2, BL], F32, tag="t3m")
            if fold:
                nc.scalar.activation(rz[:], hprz[:], AF.Sigmoid)
            else:
                scr = gsb.tile([P, 4, BL], F32, tag="scr")
                nc.vector.tensor_add(scr[:], xp_t[:, 0:4, ssl], hprz[:])
                nc.scalar.activation(rz[:], scr[:], AF.Sigmoid)
            nc.vector.tensor_mul(t1[:], rz[:, 0:2, :], hpn[:])
            nc.vector.tensor_add(sn[:], t1[:], xp_t[:, 4:6, ssl])
            # off-critical-path z*h (runs on DVE while ACT does tanh)
            nc.vector.tensor_mul(zh[:], rz[:, 2:4, :], h[cur][:])
            nc.scalar.activation(nt[:], sn[:], AF.Tanh)
            if stt:
                # h' = z*h - (z-1)*n = z*h + (1-z)*n
                nc.vector.scalar_tensor_tensor(
                    t3m[:], rz[:, 2:4, :], 1.0, nt[:],
                    op0=mybir.AluOpType.subtract, op1=mybir.AluOpType.mult)
                nc.vector.tensor_sub(h[nxt][:], zh[:], t3m[:])
            else:
                zc = gsb.tile([P, 2, BL], F32, tag="zc")
                nc.scalar.activation(zc[:], rz[:, 2:4, :], AF.Copy,
                                     scale=-1.0, bias=1.0)
                nc.vector.tensor_mul(t3m[:], nt[:], zc[:])
                nc.vector.tensor_add(h[nxt][:], t3m[:], zh[:])

        def emit_half(ci, parity):
            """Recurrence for chunk (2*ci+parity) reading xp2[parity];
            interleaved phase-1 for the next chunk into xp2[1-parity];
            prefetch DMA for chunk (2*ci+parity+2) into xin2[parity]."""
            cols = CH * BL
            if not nodma:
                nc.sync.dma_start(
                    xin2[parity][:],
                    d_xin.ap()[:, bass.ds(ci * (2 * cols) + (parity + 2) * cols, cols)])
            for s in range(CH):
                emit_step(s, xp2[parity])
                if s < n_interleave:
                    emit_p1_item(s, xin2[1 - parity], xp2[1 - parity])
                elif n_interleave == 0 and s == 0:
                    for it in range(12):
                        emit_p1_item(it, xin2[1 - parity], xp2[1 - parity])

        def emit_all():
            for t_ in (h[0],):
                nc.gpsimd.memset(t_[:], 0)
            # prologue: xin2[j] holds chunks of parity j throughout.
            nc.sync.dma_start(xin2[0][:], d_xin.ap()[:, 0:CH * BL])
            for it in range(12):
                emit_p1_item(it, xin2[0], xp2[0])
            # stage chunk 1 (consumed by half parity=0's interleaved phase-1)
            nc.sync.dma_start(xin2[1][:], d_xin.ap()[:, CH * BL:2 * CH * BL])
            with tc.For_i(
                0, halfiters,
                hint_engines=(mybir.EngineType.PE, mybir.EngineType.DVE),
            ) as ci:
                emit_half(ci, 0)
                emit_half(ci, 1)

        if reps > 1:
            with tc.For_i(0, reps, name="rep"):
                emit_all()
        else:
            emit_all()

        nc.sync.dma_start(d_out.ap()[:], h[0].rearrange("p a b -> p (a b)"))

    nc.compile()
    return nc


def _pack_v2(input, W_ih, W_hh, b_ih, b_hh, t_steps: int):
    input = np.asarray(input, np.float32)
    W_ih = np.asarray(W_ih, np.float32)
    W_hh = np.asarray(W_hh, np.float32)
    b_ih = np.asarray(b_ih, np.float32)
    b_hh = np.asarray(b_hh, np.float32)

    whhT = np.ascontiguousarray(W_hh.T)              # [H, G]
    whh = whhT.reshape(2, P, G).transpose(1, 0, 2).reshape(P, 2 * G)
    whh = np.ascontiguousarray(whh).astype(bf16)
    wih = np.ascontiguousarray(W_ih.T).astype(bf16)  # [128, 768]

    bias_full = b_hh.copy()
    bias_full[: 2 * H] += b_ih[: 2 * H]
    bmat32 = np.zeros((P, P), np.float32)
    bvec = bias_full.reshape(NBLK, P)
    bhi = bvec.astype(bf16).astype(np.float32)
    bmat32[0:NBLK, :] = bhi
    bmat32[NBLK: 2 * NBLK, :] = bvec - bhi
    biasmat = bmat32.astype(bf16)
    selmat = np.zeros((P, NBLK * BL), np.float32)
    for m in range(NBLK):
        selmat[m, BL * m: BL * (m + 1)] = 1.0
        selmat[m + NBLK, BL * m: BL * (m + 1)] = 1.0
    sel = selmat.astype(bf16)
    bihn = np.ascontiguousarray(b_ih[2 * H:].reshape(2, P).T)  # [128, 2]
    ident = np.eye(P, dtype=np.float32).astype(bf16)

    whhn = np.ascontiguousarray(-whh.astype(np.float32)).astype(bf16)
    shared = dict(whh=whh, whhn=whhn, wih=wih, biasmat=biasmat, sel=sel,
                  bihn=bihn, ident=ident)
    pad = 2 * CH * BL
    in_maps = []
    for c in range(NCORES):
        xs = input[c * BL: (c + 1) * BL, :t_steps, :]     # [16, t, 128]
        xt = np.ascontiguousarray(xs.transpose(2, 1, 0))  # [128, t, 16]
        xt = xt.reshape(P, t_steps * BL).astype(bf16)
        xin = np.zeros((P, t_steps * BL + pad), bf16)
        xin[:, :t_steps * BL] = xt
        m = dict(shared)
        m["xin"] = xin
        in_maps.append(m)
    return in_maps


def _unpack_v2(results):
    out = np.empty((B, H), np.float32)
    for c in range(NCORES):
        o = results[c]["hout"].astype(np.float32).reshape(P, 2, BL)
        out[c * BL: (c + 1) * BL, :] = o.transpose(2, 1, 0).reshape(BL, H)
    return out


def _build_v4(t_steps: int, reps: int = 1, n_interleave: int = 12,
              evac_split: int = 1):
    """v4 = v2 + (F) r|z PSUM groups split so sigmoid_r starts sooner, and
    (G) split-feed: next-step matmuls consume bf16 z*h and (z-1)*n streams
    (negated weight copy for the subtraction), so the final h subtract is
    off the critical recurrence cycle.

    Critical cycle per step: STT -> 4 r-side t3m matmuls + stop -> psum
    drain -> ACT sigmoid_r -> DVE mul/add -> ACT tanh -> STT.
    """
    nchunks = t_steps // CH
    assert nchunks % 2 == 0
    halfiters = nchunks // 2
    nc = bacc.Bacc(
        "TRN2", target_bir_lowering=False, debug=False, num_devices=NCORES
    )

    d_xin = nc.dram_tensor("xin", [P, (t_steps + 2 * CH) * BL], BF16,
                           kind="ExternalInput")
    d_whh = nc.dram_tensor("whh", [P, 2 * G], BF16, kind="ExternalInput")
    d_whhn = nc.dram_tensor("whhn", [P, 2 * G], BF16, kind="ExternalInput")
    d_wih = nc.dram_tensor("wih", [P, G], BF16, kind="ExternalInput")
    d_biasmat = nc.dram_tensor("biasmat", [P, P], BF16, kind="ExternalInput")
    d_sel = nc.dram_tensor("sel", [P, NBLK * BL], BF16, kind="ExternalInput")
    d_bihn = nc.dram_tensor("bihn", [P, 2], F32, kind="ExternalInput")
    d_ident = nc.dram_tensor("ident", [P, P], BF16, kind="ExternalInput")
    d_out = nc.dram_tensor("hout", [P, 2 * BL], BF16, kind="ExternalOutput")

    with tile.TileContext(nc) as tc, ExitStack() as ctx:
        cpool = ctx.enter_context(tc.tile_pool(name="const", bufs=1))
        state = ctx.enter_context(tc.tile_pool(name="state", bufs=1))
        gsb = ctx.enter_context(tc.tile_pool(name="gates", bufs=2))
        phr_p = ctx.enter_context(tc.tile_pool(name="phr", bufs=2, space="PSUM"))
        phz_p = ctx.enter_context(tc.tile_pool(name="phz", bufs=2, space="PSUM"))
        phpn = ctx.enter_context(tc.tile_pool(name="phpn", bufs=2, space="PSUM"))
        px = ctx.enter_context(tc.tile_pool(name="px", bufs=2, space="PSUM"))

        whh = cpool.tile([P, 2 * G], BF16, tag="whh")
        whhn = cpool.tile([P, 2 * G], BF16, tag="whhn")
        wih = cpool.tile([P, G], BF16, tag="wih")
        biasmat = cpool.tile([P, P], BF16, tag="biasmat")
        sel = cpool.tile([P, NBLK * BL], BF16, tag="sel")
        bihn = cpool.tile([P, 2], F32, tag="bihn")
        ident = cpool.tile([P, P], BF16, tag="ident")
        for dst, src in ((whh, d_whh), (whhn, d_whhn), (wih, d_wih),
                         (biasmat, d_biasmat), (sel, d_sel), (bihn, d_bihn),
                         (ident, d_ident)):
            nc.sync.dma_start(dst[:], src.ap()[:])

        h = [state.tile([P, 2, BL], BF16, name=f"h{j}", tag=f"h{j}")
             for j in range(2)]
        zh2 = [state.tile([P, 2, BL], BF16, name=f"zh{j}", tag=f"zh{j}")
               for j in range(2)]
        t3m2 = [state.tile([P, 2, BL], BF16, name=f"t3m{j}", tag=f"t3m{j}")
                for j in range(2)]
        xp2 = [state.tile([P, NBLK, CH * BL], BF16, name=f"xp{j}", tag=f"xp{j}")
               for j in range(2)]
        xin2 = [state.tile([P, CH * BL], BF16, name=f"xin{j}", tag=f"xin{j}")
                for j in range(2)]

        def emit_p1_item(item, xin_t, xp_t):
            m, hf = item // 2, item % 2
            pxm = px.tile([P, 512], F32, tag="pxm")
            wsl = slice(128 * m, 128 * (m + 1))
            xsl = slice(512 * hf, 512 * (hf + 1))
            nc.tensor.matmul(pxm[:], wih[:, wsl], xin_t[:, xsl],
                             start=True, stop=True)
            if m < 4:
                nc.vector.tensor_copy(xp_t[:, m, xsl], pxm[:])
            else:
                nc.scalar.activation(xp_t[:, m, xsl], pxm[:], AF.Identity,
                                     bias=bihn[:, m - 4: m - 3])

        def emit_step(s, xp_t):
            cur, nxt = s % 2, (s + 1) % 2
            phr = phr_p.tile([P, 2, BL], F32, tag="phr")
            phz = phz_p.tile([P, 2, BL], F32, tag="phz")
            hpn = phpn.tile([P, 2, BL], F32, tag="hpn")
            ssl = slice(BL * s, BL * (s + 1))

            def hgroup(tgt, ms, selsl, foldsl):
                # opener: bias; then zh mms + xp fold (available early);
                # t3m mms LAST so the group closes right after STT.
                nc.tensor.matmul(tgt.rearrange("p a b -> p (a b)"),
                                 biasmat[:], sel[:, selsl],
                                 start=True, stop=False)
                for k in range(2):
                    for mi, m in enumerate(ms):
                        wsl = slice(G * k + 128 * m, G * k + 128 * (m + 1))
                        nc.tensor.matmul(tgt[:, mi, :], whh[:, wsl],
                                         zh2[cur][:, k, :],
                                         start=False, stop=False)
                if foldsl is not None:
                    nc.tensor.matmul(tgt[:], ident[:], xp_t[:, foldsl, ssl],
                                     start=False, stop=False)
                for k in range(2):
                    for mi, m in enumerate(ms):
                        wsl = slice(G * k + 128 * m, G * k + 128 * (m + 1))
                        nc.tensor.matmul(tgt[:, mi, :], whhn[:, wsl],
                                         t3m2[cur][:, k, :],
                                         start=False,
                                         stop=(k == 1 and mi == len(ms) - 1))

            hgroup(phr, (0, 1), slice(0, 2 * BL), slice(0, 2))
            hgroup(phz, (2, 3), slice(2 * BL, 4 * BL), slice(2, 4))
            hgroup(hpn, (4, 5), slice(4 * BL, 6 * BL), None)

            rt = gsb.tile([P, 2, BL], F32, tag="rt")
            zt = gsb.tile([P, 2, BL], F32, tag="zt")
            t1 = gsb.tile([P, 2, BL], F32, tag="t1")
            sn = gsb.tile([P, 2, BL], F32, tag="sn")
            nt = gsb.tile([P, 2, BL], F32, tag="nt")
            nc.scalar.activation(rt[:], phr[:], AF.Sigmoid)
            nc.scalar.activation(zt[:], phz[:], AF.Sigmoid)
            nc.vector.tensor_mul(t1[:], rt[:], hpn[:])
            nc.vector.tensor_add(sn[:], t1[:], xp_t[:, 4:6, ssl])
            # z*h for the NEXT step's feed (off the critical cycle)
            nc.vector.tensor_mul(zh2[nxt][:], zt[:], h[cur][:])
            nc.scalar.activation(nt[:], sn[:], AF.Tanh)
            nc.vector.scalar_tensor_tensor(
                t3m2[nxt][:], zt[:], 1.0, nt[:],
                op0=mybir.AluOpType.subtract, op1=mybir.AluOpType.mult)
            # materialize h for the next z*h and the final output (off-cycle)
            nc.vector.tensor_sub(h[nxt][:], zh2[nxt][:], t3m2[nxt][:])

        def emit_half(ci, parity):
            cols = CH * BL
            nc.sync.dma_start(
                xin2[parity][:],
                d_xin.ap()[:, bass.ds(ci * (2 * cols) + (parity + 2) * cols, cols)])
            for s in range(CH):
                emit_step(s, xp2[parity])
                if s < n_interleave:
                    emit_p1_item(s, xin2[1 - parity], xp2[1 - parity])

        def emit_all():
            for t_ in (h[0], zh2[0], t3m2[0]):
                nc.gpsimd.memset(t_[:], 0)
            nc.sync.dma_start(xin2[0][:], d_xin.ap()[:, 0:CH * BL])
            for it in range(12):
                emit_p1_item(it, xin2[0], xp2[0])
            nc.sync.dma_start(xin2[1][:], d_xin.ap()[:, CH * BL:2 * CH * BL])
            with tc.For_i(
                0, halfiters,
                hint_engines=(mybir.EngineType.PE, mybir.EngineType.DVE),
            ) as ci:
                emit_half(ci, 0)
                emit_half(ci, 1)

        if reps > 1:
            with tc.For_i(0, reps, name="rep"):
                emit_all()
        else:
            emit_all()

        nc.sync.dma_start(d_out.ap()[:], h[0].rearrange("p a b -> p (a b)"))

    nc.compile()
    return nc


# ---------------------------------------------------------------------------
# v3: fully-DVE gate chain via custom polynomial activation micro-ops.
# sigma_z stays exact on ACT (parallel, off critical path); z*h on GPSIMD.
# tanh ~ clamp(x*P6(x^2)) (3 DVE insts), r folded into t1 via
# clamp(x*Q4+0.5,0,1)*hpn (3 DVE insts incl. the multiply).
# ---------------------------------------------------------------------------

# minimax coefficients: tanh ~ clamp(x*P6(x^2)) fit on [0,3.6] (global err
# 2.2e-3), sigma ~ clamp(0.5 + x*Q4(x^2), 0, 1) fit on [0,6.0] (err 2.7e-3).
_CT = [0.98913864512, -0.28780500403, 0.073307507501, -0.012014940477,
       0.0011548376344, -5.8764841675e-05, 1.21686344e-06]
_CS = [0.24396367529, -0.015843419093, 0.00076713848496, -1.9324037226e-05,
       1.8881984989e-07]

_POLY_OPS: dict = {}


def _register_poly_ops():
    """Define + register the custom DVE ops at runtime (repo is read-only;
    the documented flow is appending to dve_ops.OPS)."""
    if _POLY_OPS:
        return _POLY_OPS
    import numpy as _np
    import concourse.dve_ops as _dops
    from concourse.dve_spec import (
        Spec, Src0, Src1, C0, C1, C2, Zero, One, sq, minn, maxx, lower,
        _has_src1,
    )
    from concourse.dve_table_gen import dve_ver_for
    from concourse.dve_uop import DveOpSpec

    t = sq(Src0)
    bodies = {
        # w = ((c_a*t + c_b)*t + c_c)*t
        "ANT_POLY1": (
            ((C0 * t + C1) * t + C2) * t,
            lambda in0, in1, s0, s1, imm2:
                (((s0 * (in0 * in0) + s1) * (in0 * in0) + imm2)
                 * (in0 * in0)).astype(_np.float32)),
        # out = ((w + c_a)*t + c_b)*t
        "ANT_POLY2T": (
            ((Src1 + C0) * t + C1) * t,
            lambda in0, in1, s0, s1, imm2:
                (((in1 + s0) * (in0 * in0) + s1)
                 * (in0 * in0)).astype(_np.float32)),
        # out = ((w + c_a)*t + c_b)*x  (sigma pre-clamp)
        "ANT_POLY2X": (
            ((Src1 + C0) * t + C1) * Src0,
            lambda in0, in1, s0, s1, imm2:
                (((in1 + s0) * (in0 * in0) + s1) * in0).astype(_np.float32)),
        # out = clamp(y + c0, 0, 1) * in1  (finish sigma, multiply by hpn)
        "ANT_SIGFIN_MUL": (
            maxx(minn(Src0 + C0, One), Zero) * Src1,
            lambda in0, in1, s0, s1, imm2:
                (_np.clip(in0 + s0, 0.0, 1.0) * in1).astype(_np.float32)),
        # out = clamp(((w + c_a)*t + c_b)*x, -1, 1)  (finish tanh)
        "ANT_POLY3C": (
            maxx(minn(((Src1 + C0) * t + C1) * Src0, One), Zero - One),
            lambda in0, in1, s0, s1, imm2:
                _np.clip(((in1 + s0) * (in0 * in0) + s1) * in0,
                         -1.0, 1.0).astype(_np.float32)),
    }
    base = max(_dops._SUB_OPCODE_FOR_NAME.values()) + 1
    for i, (name, (body, ref)) in enumerate(bodies.items()):
        if name in _dops._SUB_OPCODE_FOR_NAME:
            _POLY_OPS[name] = next(o for o in _dops.OPS if o.name == name)
            continue
        spec = Spec(body=body, reference=ref)
        shas = {}
        for ver in ("v3", "v4"):
            u = lower(spec, ver=ver)
            shas[ver] = DveOpSpec(name=name, uops=u,
                                  rd1_en=_has_src1(spec)).sha(ver)
        op = _dops.DveOp(name, spec, subdim=False, uops_sha=shas)
        _dops._SUB_OPCODE_FOR_NAME[name] = base + i
        _dops.OPS.append(op)
        _dops.CUSTOM_DVE_SPECS[name] = spec
        _POLY_OPS[name] = op
    return _POLY_OPS


def _build_v3(t_steps: int, reps: int = 1, n_interleave: int = 12):
    ops = _register_poly_ops()
    P1, P2T, P2X, SFM, P3C = (ops["ANT_POLY1"], ops["ANT_POLY2T"],
                              ops["ANT_POLY2X"], ops["ANT_SIGFIN_MUL"],
                              ops["ANT_POLY3C"])
    ct, cs = _CT, _CS
    nchunks = t_steps // CH
    assert nchunks % 2 == 0
    halfiters = nchunks // 2
    nc = bacc.Bacc(
        "TRN2", target_bir_lowering=False, debug=False, num_devices=NCORES
    )

    d_xin = nc.dram_tensor("xin", [P, (t_steps + 2 * CH) * BL], BF16,
                           kind="ExternalInput")
    d_whh = nc.dram_tensor("whh", [P, 2 * G], BF16, kind="ExternalInput")
    d_wih = nc.dram_tensor("wih", [P, G], BF16, kind="ExternalInput")
    d_biasmat = nc.dram_tensor("biasmat", [P, P], BF16, kind="ExternalInput")
    d_sel = nc.dram_tensor("sel", [P, NBLK * BL], BF16, kind="ExternalInput")
    d_bihn = nc.dram_tensor("bihn", [P, 2], F32, kind="ExternalInput")
    d_ident = nc.dram_tensor("ident", [P, P], BF16, kind="ExternalInput")
    d_out = nc.dram_tensor("hout", [P, 2 * BL], BF16, kind="ExternalOutput")

    with tile.TileContext(nc) as tc, ExitStack() as ctx:
        cpool = ctx.enter_context(tc.tile_pool(name="const", bufs=1))
        state = ctx.enter_context(tc.tile_pool(name="state", bufs=1))
        gsb = ctx.enter_context(tc.tile_pool(name="gates", bufs=2))
        php = ctx.enter_context(tc.tile_pool(name="php", bufs=2, space="PSUM"))
        phpn = ctx.enter_context(tc.tile_pool(name="phpn", bufs=2, space="PSUM"))
        px = ctx.enter_context(tc.tile_pool(name="px", bufs=2, space="PSUM"))

        whh = cpool.tile([P, 2 * G], BF16, tag="whh")
        wih = cpool.tile([P, G], BF16, tag="wih")
        biasmat = cpool.tile([P, P], BF16, tag="biasmat")
        sel = cpool.tile([P, NBLK * BL], BF16, tag="sel")
        bihn = cpool.tile([P, 2], F32, tag="bihn")
        ident = cpool.tile([P, P], BF16, tag="ident")
        for dst, src in ((whh, d_whh), (wih, d_wih), (biasmat, d_biasmat),
                         (sel, d_sel), (bihn, d_bihn), (ident, d_ident)):
            nc.sync.dma_start(dst[:], src.ap()[:])

        h = [state.tile([P, 2, BL], BF16, name=f"h{j}", tag=f"h{j}")
             for j in range(2)]
        xp2 = [state.tile([P, NBLK, CH * BL], BF16, name=f"xp{j}", tag=f"xp{j}")
               for j in range(2)]
        xin2 = [state.tile([P, CH * BL], BF16, name=f"xin{j}", tag=f"xin{j}")
                for j in range(2)]

        def emit_p1_item(item, xin_t, xp_t):
            m, hf = item // 2, item % 2
            pxm = px.tile([P, 512], F32, tag="pxm")
            wsl = slice(128 * m, 128 * (m + 1))
            xsl = slice(512 * hf, 512 * (hf + 1))
            nc.tensor.matmul(pxm[:], wih[:, wsl], xin_t[:, xsl],
                             start=True, stop=True)
            if m < 4:
                nc.vector.tensor_copy(xp_t[:, m, xsl], pxm[:])
            else:
                nc.scalar.activation(xp_t[:, m, xsl], pxm[:], AF.Identity,
                                     bias=bihn[:, m - 4: m - 3])

        def emit_step(s, xp_t):
            cur, nxt = s % 2, (s + 1) % 2
            hprz = php.tile([P, 4, BL], F32, tag="hprz")
            hpn = phpn.tile([P, 2, BL], F32, tag="hpn")
            ssl = slice(BL * s, BL * (s + 1))
            nc.tensor.matmul(hprz.rearrange("p a b -> p (a b)"),
                             biasmat[:], sel[:, 0:4 * BL],
                             start=True, stop=False)
            for k in range(2):
                for m in range(4):
                    wsl = slice(G * k + 128 * m, G * k + 128 * (m + 1))
                    nc.tensor.matmul(hprz[:, m, :], whh[:, wsl], h[cur][:, k, :],
                                     start=False, stop=False)
            nc.tensor.matmul(hprz[:], ident[:], xp_t[:, 0:4, ssl],
                             start=False, stop=True)
            nc.tensor.matmul(hpn.rearrange("p a b -> p (a b)"),
                             biasmat[:], sel[:, 4 * BL:6 * BL],
                             start=True, stop=False)
            for k in range(2):
                for mi, m in enumerate((4, 5)):
                    wsl = slice(G * k + 128 * m, G * k + 128 * (m + 1))
                    nc.tensor.matmul(hpn[:, mi, :], whh[:, wsl], h[cur][:, k, :],
                                     start=False, stop=(k == 1 and mi == 1))

            hr = hprz[:, 0:2, :].rearrange("p a b -> p (a b)")   # [P,32] PSUM
            hz = hprz[:, 2:4, :].rearrange("p a b -> p (a b)")
            hpn_f = hpn.rearrange("p a b -> p (a b)")
            F = 2 * BL
            w1 = gsb.tile([P, F], F32, tag="w1")
            y2 = gsb.tile([P, F], F32, tag="y2")
            t1 = gsb.tile([P, F], F32, tag="t1")
            sn = gsb.tile([P, F], F32, tag="sn")
            tw1 = gsb.tile([P, F], F32, tag="tw1")
            tw2 = gsb.tile([P, F], F32, tag="tw2")
            nt = gsb.tile([P, F], F32, tag="nt")
            zt = gsb.tile([P, F], F32, tag="zt")
            zh = gsb.tile([P, F], F32, tag="zh")
            t3m = gsb.tile([P, F], F32, tag="t3m")
            # r/n chain, all DVE. Emitted BEFORE the ACT/Pool z-path: tile
            # readers of the same psum tile are chained in emission order,
            # so the critical DVE chain must read hprz first.
            nc.vector._custom_dve(P1, out=w1[:], in0=hr,
                                  s0=cs[4], s1=cs[3], imm2=cs[2])
            nc.vector._custom_dve(P2X, out=y2[:], in0=hr, in1=w1[:],
                                  s0=cs[1], s1=cs[0])
            # z path: exact sigmoid on ACT, z*h on GPSIMD — both parallel to
            # the rest of the DVE chain.
            nc.scalar.activation(zt[:], hz, AF.Sigmoid)
            nc.gpsimd.tensor_mul(zh[:], zt[:],
                                 h[cur].rearrange("p a b -> p (a b)"))
            nc.vector._custom_dve(SFM, out=t1[:], in0=y2[:], in1=hpn_f,
                                  s0=0.5)
            nc.vector.tensor_add(sn[:], t1[:], xp_t[:, 4:6, ssl])
            nc.vector._custom_dve(P1, out=tw1[:], in0=sn[:],
                                  s0=ct[6], s1=ct[5], imm2=ct[4])
            nc.vector._custom_dve(P2T, out=tw2[:], in0=sn[:], in1=tw1[:],
                                  s0=ct[3], s1=ct[2])
            nc.vector._custom_dve(P3C, out=nt[:], in0=sn[:], in1=tw2[:],
                                  s0=ct[1], s1=ct[0])
            nc.vector.scalar_tensor_tensor(
                t3m[:], zt[:], 1.0, nt[:],
                op0=mybir.AluOpType.subtract, op1=mybir.AluOpType.mult)
            nc.vector.tensor_sub(h[nxt].rearrange("p a b -> p (a b)"),
                                 zh[:], t3m[:])

        def emit_half(ci, parity):
            cols = CH * BL
            nc.sync.dma_start(
                xin2[parity][:],
                d_xin.ap()[:, bass.ds(ci * (2 * cols) + (parity + 2) * cols, cols)])
            for s in range(CH):
                emit_step(s, xp2[parity])
                if s < n_interleave:
                    emit_p1_item(s, xin2[1 - parity], xp2[1 - parity])

        def emit_all():
            nc.gpsimd.memset(h[0][:], 0)
            nc.sync.dma_start(xin2[0][:], d_xin.ap()[:, 0:CH * BL])
            for it in range(12):
                emit_p1_item(it, xin2[0], xp2[0])
            nc.sync.dma_start(xin2[1][:], d_xin.ap()[:, CH * BL:2 * CH * BL])
            with tc.For_i(
                0, halfiters,
                hint_engines=(mybir.EngineType.PE, mybir.EngineType.DVE),
            ) as ci:
                emit_half(ci, 0)
                emit_half(ci, 1)

        if reps > 1:
            with tc.For_i(0, reps, name="rep"):
                emit_all()
        else:
            emit_all()

        nc.sync.dma_start(d_out.ap()[:], h[0].rearrange("p a b -> p (a b)"))

    nc.compile()
    return nc


_PROGRAM_CACHE: dict = {}


def _get_program(t_steps: int, reps: int = 1, ver: int = 2):
    key = (t_steps, reps, ver)
    if key not in _PROGRAM_CACHE:
        builder = {1: _build_program, 2: _build_v2, 3: _build_v3,
                   4: _build_v4}[ver]
        _PROGRAM_CACHE[key] = builder(t_steps, reps)
    return _PROGRAM_CACHE[key]


def _pack_inputs(input, W_ih, W_hh, b_ih, b_hh, t_steps: int):
    """Host-side packing. Returns per-core in_maps."""
    input = np.asarray(input, np.float32)
    W_ih = np.asarray(W_ih, np.float32)
    W_hh = np.asarray(W_hh, np.float32)
    b_ih = np.asarray(b_ih, np.float32)
    b_hh = np.asarray(b_hh, np.float32)

    # weights, feature-major packed (shared by all cores)
    whhT = np.ascontiguousarray(W_hh.T)              # [H, G]
    whh = whhT.reshape(2, P, G).transpose(1, 0, 2).reshape(P, 2 * G)
    whh_hi, whh_lo = _split_hi_lo(np.ascontiguousarray(whh))
    wihT = np.ascontiguousarray(W_ih.T)              # [X, G] = [128, 768]
    wih_hi, wih_lo = _split_hi_lo(wihT)

    # bias matrix: rows 0..5 hi parts, rows 6..11 lo parts; selector picks both
    bias_full = b_hh.copy()
    bias_full[: 2 * H] += b_ih[: 2 * H]              # r,z: b_ih + b_hh; n: b_hh
    bmat32 = np.zeros((P, P), np.float32)
    bvec = bias_full.reshape(NBLK, P)
    bhi = bvec.astype(bf16).astype(np.float32)
    blo = bvec - bhi
    bmat32[0:NBLK, :] = bhi
    bmat32[NBLK: 2 * NBLK, :] = blo
    biasmat = bmat32.astype(bf16)
    selmat = np.zeros((P, NBLK * BL), np.float32)
    for m in range(NBLK):
        selmat[m, BL * m: BL * (m + 1)] = 1.0
        selmat[m + NBLK, BL * m: BL * (m + 1)] = 1.0
    sel = selmat.astype(bf16)
    bihn = np.ascontiguousarray(b_ih[2 * H:].reshape(2, P).T)  # [128, 2]

    shared = dict(
        whh_hi=whh_hi, whh_lo=whh_lo, wih_hi=wih_hi, wih_lo=wih_lo,
        biasmat=biasmat, sel=sel, bihn=bihn,
        whh_f=np.ascontiguousarray(whh), biasmat_f=bmat32, sel_f=selmat,
    )
    in_maps = []
    for c in range(NCORES):
        xs = input[c * BL: (c + 1) * BL, :t_steps, :]     # [16, t, 128]
        xt = np.ascontiguousarray(xs.transpose(2, 1, 0))  # [128, t, 16]
        xt = xt.reshape(P, t_steps * BL)
        xh, xl = _split_hi_lo(xt)
        m = dict(shared)
        m["xin_hi"] = xh
        m["xin_lo"] = xl
        in_maps.append(m)
    return in_maps


def _unpack_output(results):
    out = np.empty((B, H), np.float32)
    for c in range(NCORES):
        o = results[c]["hout"].reshape(P, 2, BL)           # [p, k, b]
        out[c * BL: (c + 1) * BL, :] = o.transpose(2, 1, 0).reshape(BL, H)
    return out


VER = 2


def run(input, W_ih, W_hh, b_ih, b_hh, t_steps: int = T, trace: bool = False,
        ver: int = None):
    ver = VER if ver is None else ver
    nc = _get_program(t_steps, ver=ver)
    pack = _pack_inputs if ver == 1 else _pack_v2
    unpack = _unpack_output if ver == 1 else _unpack_v2
    in_maps = pack(input, W_ih, W_hh, b_ih, b_hh, t_steps)
    res = run_bass_kernel_spmd(
        nc, in_maps, core_ids=list(range(NCORES)), trace=trace
    )
    return unpack(res.results), res


def kernel(input, W_ih, W_hh, b_ih, b_hh):
    out, _ = run(input, W_ih, W_hh, b_ih, b_hh)
    return out


def bench(input, W_ih, W_hh, b_ih, b_hh, reps_hi: int = 5, iters: int = 3,
          ver: int = None):
    """Estimate on-device time: wall(R=reps_hi) - wall(R=1) over cached
    executables, divided by (reps_hi - 1). Returns ns."""
    import time as _time

    ver = VER if ver is None else ver
    pack = _pack_v2 if ver == 2 else _pack_inputs
    in_maps = pack(input, W_ih, W_hh, b_ih, b_hh, T)
    nc1 = _get_program(T, 1, ver=ver)
    ncR = _get_program(T, reps_hi, ver=ver)

    def timed(nc):
        best = float("inf")
        for _ in range(iters):
            t0 = _time.perf_counter()
            run_bass_kernel_spmd(nc, in_maps, core_ids=list(range(NCORES)))
            best = min(best, _time.perf_counter() - t0)
        return best

    # warm both executables (compile cache)
    run_bass_kernel_spmd(nc1, in_maps, core_ids=list(range(NCORES)))
    run_bass_kernel_spmd(ncR, in_maps, core_ids=list(range(NCORES)))
    t1 = timed(nc1)
    tR = timed(ncR)
    ns = (tR - t1) / (reps_hi - 1) * 1e9
    print(f"wall R=1: {t1*1e3:.1f} ms   wall R={reps_hi}: {tR*1e3:.1f} ms")
    return ns



# revision 23
# speedup vs baseline: 62.5508x; 4.7524x over previous
"""GRU encoder (nn_Encoder_26087631356042) Bass/Trainium2 kernel.

Strategy: data-parallel over batch (B=128 -> 16 per core, 8 cores, no
collectives). Per core, a fused kernel: the input projection GEMM
(x @ W_ih.T) is computed 32 timesteps at a time inside the sequential
GRU time loop, entirely in feature-major "packed" layout
(feature f -> (block m = f//128, partition p = f%128)), so all gate
elementwise ops run with 128 active partitions and tiny free dims.

The recurrent matmul keeps W_hh.T stationary (bf16 hi+lo split) and
streams the hidden state (bf16 hi+lo split) as the moving operand,
accumulating exactly in fp32 PSUM; biases enter through a rank-1
"bias matmul" that also serves as the accumulation-group opener.
"""

import os
import numpy as np
import ml_dtypes
from contextlib import ExitStack

import concourse.bass as bass
import concourse.bacc as bacc
import concourse.tile as tile
import concourse.mybir as mybir
from concourse.bass_utils import run_bass_kernel_spmd

F32 = mybir.dt.float32
BF16 = mybir.dt.bfloat16
AF = mybir.ActivationFunctionType

B, T, X, H = 128, 2048, 128, 256
G = 3 * H          # 768 gate features
NBLK = G // 128    # 6 feature blocks
NCORES = 8
BL = B // NCORES   # 16 batch rows per core
CH = 64            # timesteps per For_i body
P = 128

bf16 = ml_dtypes.bfloat16


def _split_hi_lo(a32: np.ndarray):
    hi = a32.astype(bf16)
    lo = (a32 - hi.astype(np.float32)).astype(bf16)
    return hi, lo


def _build_program(t_steps: int, reps: int = 1, nogates: bool = False,
                   single: bool = False, nophase1: bool = False,
                   f32r_rhs: bool = False, allr: bool = False,
                   coltile: bool = False):
    """Emit the per-core program (same program on all cores; data differs).

    reps > 1 wraps the whole computation in an outer repeat loop (state
    carries over between reps — outputs are only timing-valid).
    nogates/single/nophase1 are timing-ablation variants."""
    nchunks = t_steps // CH
    nc = bacc.Bacc(
        "TRN2", target_bir_lowering=False, debug=False, num_devices=NCORES
    )

    # DRAM I/O
    d_xin_hi = nc.dram_tensor("xin_hi", [P, t_steps * BL], BF16, kind="ExternalInput")
    d_xin_lo = nc.dram_tensor("xin_lo", [P, t_steps * BL], BF16, kind="ExternalInput")
    d_whh_hi = nc.dram_tensor("whh_hi", [P, 2 * G], BF16, kind="ExternalInput")
    d_whh_lo = nc.dram_tensor("whh_lo", [P, 2 * G], BF16, kind="ExternalInput")
    d_wih_hi = nc.dram_tensor("wih_hi", [P, G], BF16, kind="ExternalInput")
    d_wih_lo = nc.dram_tensor("wih_lo", [P, G], BF16, kind="ExternalInput")
    d_biasmat = nc.dram_tensor("biasmat", [P, P], BF16, kind="ExternalInput")
    d_sel = nc.dram_tensor("sel", [P, NBLK * BL], BF16, kind="ExternalInput")
    d_bihn = nc.dram_tensor("bihn", [P, 2], F32, kind="ExternalInput")
    if allr:
        d_whh_f = nc.dram_tensor("whh_f", [P, 2 * G], F32, kind="ExternalInput")
        d_biasmat_f = nc.dram_tensor("biasmat_f", [P, P], F32, kind="ExternalInput")
        d_sel_f = nc.dram_tensor("sel_f", [P, NBLK * BL], F32, kind="ExternalInput")
    d_out = nc.dram_tensor("hout", [P, 2 * BL], F32, kind="ExternalOutput")

    with tile.TileContext(nc) as tc, ExitStack() as ctx:
        cpool = ctx.enter_context(tc.tile_pool(name="const", bufs=1))
        state = ctx.enter_context(tc.tile_pool(name="state", bufs=1))
        xpp = ctx.enter_context(tc.tile_pool(name="xp", bufs=1))
        xinp = ctx.enter_context(tc.tile_pool(name="xin", bufs=2))
        gsb = ctx.enter_context(tc.tile_pool(name="gates", bufs=2))
        php = ctx.enter_context(tc.tile_pool(name="php", bufs=2, space="PSUM"))
        phpn = ctx.enter_context(tc.tile_pool(name="phpn", bufs=2, space="PSUM"))
        pscr = ctx.enter_context(tc.tile_pool(name="pscr", bufs=2, space="PSUM"))
        px = ctx.enter_context(tc.tile_pool(name="px", bufs=2, space="PSUM"))

        # Constants -> SBUF
        whh_hi = cpool.tile([P, 2 * G], BF16, tag="whh_hi")
        whh_lo = cpool.tile([P, 2 * G], BF16, tag="whh_lo")
        wih_hi = cpool.tile([P, G], BF16, tag="wih_hi")
        wih_lo = cpool.tile([P, G], BF16, tag="wih_lo")
        biasmat = cpool.tile([P, P], BF16, tag="biasmat")
        sel = cpool.tile([P, NBLK * BL], BF16, tag="sel")
        bihn = cpool.tile([P, 2], F32, tag="bihn")
        loads = [
            (whh_hi, d_whh_hi), (whh_lo, d_whh_lo),
            (wih_hi, d_wih_hi), (wih_lo, d_wih_lo),
            (biasmat, d_biasmat), (sel, d_sel), (bihn, d_bihn),
        ]
        if allr:
            whh_f = cpool.tile([P, 2 * G], F32, tag="whh_f")
            biasmat_f = cpool.tile([P, P], F32, tag="biasmat_f")
            sel_f = cpool.tile([P, NBLK * BL], F32, tag="sel_f")
            loads += [(whh_f, d_whh_f), (biasmat_f, d_biasmat_f), (sel_f, d_sel_f)]
            whh_r = whh_f.bitcast(mybir.dt.float32r)
            biasmat_r = biasmat_f.bitcast(mybir.dt.float32r)
            sel_r = sel_f.bitcast(mybir.dt.float32r)
        for dst, src in loads:
            nc.sync.dma_start(dst[:], src.ap()[:])

        # Hidden state (feature-major packed): [128, 2 k-blocks, 16 batch]
        # Ping-pong pairs; CH is even so every body starts and ends on idx 0.
        hT = [state.tile([P, 2, BL], F32, name=f"hT{j}", tag=f"hT{j}") for j in range(2)]
        hTr = [t.bitcast(mybir.dt.float32r) for t in hT]
        hhi = [state.tile([P, 2, BL], BF16, name=f"hhi{j}", tag=f"hhi{j}") for j in range(2)]
        hlo = [state.tile([P, 2, BL], BF16, name=f"hlo{j}", tag=f"hlo{j}") for j in range(2)]
        for t_ in (hT[0], hhi[0], hlo[0]):
            nc.gpsimd.memset(t_[:], 0)

        # xp slab for one chunk: [128, 6 blocks, CH*BL cols] fp32
        xp = xpp.tile([P, NBLK, CH * BL], F32, tag="xp")
        if nophase1:
            nc.gpsimd.memset(xp[:], 0)

        def emit_time_loop():
          with tc.For_i(
            0, nchunks,
            hint_engines=(mybir.EngineType.PE, mybir.EngineType.DVE),
          ) as ci:
            # ---- Phase 1: xp = Wih @ x for CH steps (feature-major) ----
            xh = xinp.tile([P, CH * BL], BF16, tag="xh")
            xl = xinp.tile([P, CH * BL], BF16, tag="xl")
            nc.sync.dma_start(xh[:], d_xin_hi.ap()[:, bass.ts(ci, CH * BL)])
            nc.sync.dma_start(xl[:], d_xin_lo.ap()[:, bass.ts(ci, CH * BL)])
            for m in range(NBLK if not nophase1 else 0):
                for hf in range(CH * BL // 512):
                    pxm = px.tile([P, 512], F32, tag="pxm")
                    wsl = slice(128 * m, 128 * (m + 1))
                    xsl = slice(512 * hf, 512 * (hf + 1))
                    nc.tensor.matmul(pxm[:], wih_hi[:, wsl], xh[:, xsl],
                                     start=True, stop=False)
                    nc.tensor.matmul(pxm[:], wih_hi[:, wsl], xl[:, xsl],
                                     start=False, stop=False)
                    nc.tensor.matmul(pxm[:], wih_lo[:, wsl], xh[:, xsl],
                                     start=False, stop=True)
                    if m < 4:
                        nc.vector.tensor_copy(xp[:, m, xsl], pxm[:])
                    else:
                        # fold b_ih (n-gate part) in during evacuation
                        nc.scalar.activation(
                            xp[:, m, xsl], pxm[:], AF.Identity,
                            bias=bihn[:, m - 4: m - 3],
                        )

            # ---- Recurrence over CH steps ----
            for s in range(CH):
                cur, nxt = s % 2, (s + 1) % 2
                # split psum tiles: rz completes first so the sigmoid path
                # overlaps the n-block matmuls (deps are tile-granular)
                hprz = php.tile([P, 4, BL], F32, tag="hprz")
                hpn = phpn.tile([P, 2, BL], F32, tag="hpn")
                # bias matmuls open the accumulation groups (shared lhsT)
                bm = biasmat_r if allr else biasmat
                sl = sel_r if allr else sel
                nc.tensor.matmul(hprz.rearrange("p a b -> p (a b)"),
                                 bm[:], sl[:, 0:4 * BL],
                                 start=True, stop=False)
                nc.tensor.matmul(hpn.rearrange("p a b -> p (a b)"),
                                 bm[:], sl[:, 4 * BL:],
                                 start=True, stop=False)

                def emit_mms(ms, tgt, off):
                    for mi, m in enumerate(ms):
                        for k in range(2):
                            rh = hhi[cur][:, k, :]
                            rl = hlo[cur][:, k, :]
                            wsl = slice(G * k + 128 * m, G * k + 128 * (m + 1))
                            last = (k == 1 and mi == len(ms) - 1)
                            o = tgt[:, m - off, :]
                            if allr:
                                rf = hTr[cur][:, k, :]
                                nc.tensor.matmul(o, whh_r[:, wsl], rf,
                                                 start=False, stop=last)
                                continue
                            if f32r_rhs:
                                # exact h streamed as fp32r against bf16 weights
                                rf = hTr[cur][:, k, :]
                                nc.tensor.matmul(o, whh_hi[:, wsl], rf,
                                                 start=False, stop=False)
                                nc.tensor.matmul(o, whh_lo[:, wsl], rf,
                                                 start=False, stop=last)
                                continue
                            if single:
                                nc.tensor.matmul(o, whh_hi[:, wsl], rh,
                                                 start=False, stop=last)
                                continue
                            if coltile:
                                # [128,32] weight subtiles -> 4 col-groups of
                                # the PE array load + compute concurrently
                                base = G * k + 128 * m
                                for ti, (w, r) in enumerate(
                                    ((whh_hi, rh), (whh_hi, rl), (whh_lo, rh))
                                ):
                                    for q in range(4):
                                        qs = slice(base + 32 * q,
                                                   base + 32 * (q + 1))
                                        oq = o[32 * q: 32 * (q + 1), :]
                                        nc.tensor.matmul(
                                            oq, w[:, qs], r,
                                            start=False,
                                            stop=(last and ti == 2 and q == 3),
                                            tile_position=(0, 32 * q),
                                        )
                                continue
                            nc.tensor.matmul(o, whh_hi[:, wsl], rh,
                                             start=False, stop=False)
                            nc.tensor.matmul(o, whh_hi[:, wsl], rl,
                                             start=False, stop=False)
                            nc.tensor.matmul(o, whh_lo[:, wsl], rh,
                                             start=False, stop=last)

                emit_mms((0, 1, 2, 3), hprz, 0)
                emit_mms((4, 5), hpn, 4)

                xp_t = xp[:, :, bass.ts(s, BL)]          # [128, 6, 16]
                scr = pscr.tile([P, NBLK, BL], F32, tag="scr")
                rz = gsb.tile([P, 4, BL], F32, tag="rz")
                t1 = gsb.tile([P, 2, BL], F32, tag="t1")
                nsb = gsb.tile([P, 2, BL], F32, tag="nsb")
                zc = gsb.tile([P, 2, BL], F32, tag="zc")
                zh = gsb.tile([P, 2, BL], F32, tag="zh")
                t3 = gsb.tile([P, 2, BL], F32, tag="t3")

                if nogates:
                    # ablation: keep the serial dep chain, drop gate math
                    nc.vector.tensor_copy(hT[nxt][:], hpn[:, 0:2, :])
                    nc.vector.tensor_copy(hhi[nxt][:], hT[nxt][:])
                    nc.vector.tensor_sub(hlo[nxt][:], hT[nxt][:], hhi[nxt][:])
                    continue
                # r,z pre-activations then sigmoid (overlaps n-block MMs)
                nc.vector.tensor_add(scr[:, 0:4, :], xp_t[:, 0:4, :], hprz[:])
                nc.scalar.activation(rz[:], scr[:, 0:4, :], AF.Sigmoid)
                nc.scalar.activation(zc[:], rz[:, 2:4, :], AF.Copy,
                                     scale=-1.0, bias=1.0)
                # off-critical-path: z*h
                nc.vector.tensor_mul(zh[:], rz[:, 2:4, :], hT[cur][:])
                # n = tanh(xn + r*hn)   (b_ih_n already in xp, b_hh_n in hp)
                nc.vector.tensor_mul(t1[:], rz[:, 0:2, :], hpn[:])
                nc.vector.tensor_add(scr[:, 4:6, :], t1[:], xp_t[:, 4:6, :])
                nc.scalar.activation(nsb[:], scr[:, 4:6, :], AF.Tanh)
                # h' = (1-z)*n + z*h ; emit the bf16 hi part FIRST so the
                # next step's Whi@hhi matmuls can start one op earlier
                nc.vector.tensor_mul(t3[:], nsb[:], zc[:])
                nc.vector.tensor_add(hhi[nxt][:], t3[:], zh[:])
                nc.vector.tensor_add(hT[nxt][:], t3[:], zh[:])
                nc.vector.tensor_sub(hlo[nxt][:], hT[nxt][:], hhi[nxt][:])

        if reps > 1:
            with tc.For_i(0, reps, name="rep"):
                emit_time_loop()
        else:
            emit_time_loop()

        nc.sync.dma_start(d_out.ap()[:], hT[0].rearrange("p a b -> p (a b)"))

    nc.compile()
    return nc


def _build_v2(t_steps: int, reps: int = 1, p1pool: bool = True,
              n_interleave: int = 12, act_evac: bool = True,
              fold: bool = True, stt: bool = True, nodma: bool = False):
    """v2: single-bf16 GRU step, xp folded into PSUM via identity matmul,
    STT-fused gate tail, phase-1 interleaved into the recurrence.

    Per-step serial chain: PE group (15 matmuls) -> ACT sigmoid(PSUM) ->
    DVE mul -> DVE add -> ACT tanh -> DVE STT -> DVE sub -> bf16 h'.
    """
    nchunks = t_steps // CH
    assert nchunks % 2 == 0
    halfiters = nchunks // 2
    nc = bacc.Bacc(
        "TRN2", target_bir_lowering=False, debug=False, num_devices=NCORES
    )

    # DRAM I/O (xin padded by 2 chunks so the in-loop prefetch stays in-bounds)
    d_xin = nc.dram_tensor("xin", [P, (t_steps + 2 * CH) * BL], BF16,
                           kind="ExternalInput")
    d_whh = nc.dram_tensor("whh", [P, 2 * G], BF16, kind="ExternalInput")
    d_wih = nc.dram_tensor("wih", [P, G], BF16, kind="ExternalInput")
    d_biasmat = nc.dram_tensor("biasmat", [P, P], BF16, kind="ExternalInput")
    d_sel = nc.dram_tensor("sel", [P, NBLK * BL], BF16, kind="ExternalInput")
    d_bihn = nc.dram_tensor("bihn", [P, 2], F32, kind="ExternalInput")
    d_ident = nc.dram_tensor("ident", [P, P], BF16, kind="ExternalInput")
    d_out = nc.dram_tensor("hout", [P, 2 * BL], BF16, kind="ExternalOutput")

    with tile.TileContext(nc) as tc, ExitStack() as ctx:
        cpool = ctx.enter_context(tc.tile_pool(name="const", bufs=1))
        state = ctx.enter_context(tc.tile_pool(name="state", bufs=1))
        gsb = ctx.enter_context(tc.tile_pool(name="gates", bufs=2))
        php = ctx.enter_context(tc.tile_pool(name="php", bufs=2, space="PSUM"))
        phpn = ctx.enter_context(tc.tile_pool(name="phpn", bufs=2, space="PSUM"))
        px = ctx.enter_context(tc.tile_pool(name="px", bufs=2, space="PSUM"))

        whh = cpool.tile([P, 2 * G], BF16, tag="whh")
        wih = cpool.tile([P, G], BF16, tag="wih")
        biasmat = cpool.tile([P, P], BF16, tag="biasmat")
        sel = cpool.tile([P, NBLK * BL], BF16, tag="sel")
        bihn = cpool.tile([P, 2], F32, tag="bihn")
        ident = cpool.tile([P, P], BF16, tag="ident")
        for dst, src in ((whh, d_whh), (wih, d_wih), (biasmat, d_biasmat),
                         (sel, d_sel), (bihn, d_bihn), (ident, d_ident)):
            nc.sync.dma_start(dst[:], src.ap()[:])

        # hidden state ping-pong, bf16 only
        h = [state.tile([P, 2, BL], BF16, name=f"h{j}", tag=f"h{j}")
             for j in range(2)]
        # xp slabs ping-pong (bf16), xin staging ping-pong
        xp2 = [state.tile([P, NBLK, CH * BL], BF16, name=f"xp{j}", tag=f"xp{j}")
               for j in range(2)]
        xin2 = [state.tile([P, CH * BL], BF16, name=f"xin{j}", tag=f"xin{j}")
                for j in range(2)]

        def emit_p1_item(item, xin_t, xp_t):
            """Phase-1 item `item` in 0..11: matmul (m, hf) + evacuation.
            GPSIMD can't read PSUM, so evacuate on DVE (plain copies) and
            ACT (the two n-blocks that fold in b_ih_n)."""
            m, hf = item // 2, item % 2
            pxm = px.tile([P, 512], F32, tag="pxm")
            wsl = slice(128 * m, 128 * (m + 1))
            xsl = slice(512 * hf, 512 * (hf + 1))
            nc.tensor.matmul(pxm[:], wih[:, wsl], xin_t[:, xsl],
                             start=True, stop=True)
            if m < 4:
                nc.vector.tensor_copy(xp_t[:, m, xsl], pxm[:])
            elif act_evac:
                nc.scalar.activation(xp_t[:, m, xsl], pxm[:], AF.Identity,
                                     bias=bihn[:, m - 4: m - 3])
            else:
                nc.vector.tensor_scalar_add(xp_t[:, m, xsl], pxm[:],
                                            bihn[:, m - 4: m - 3])

        def emit_step(s, xp_t):
            cur, nxt = s % 2, (s + 1) % 2
            hprz = php.tile([P, 4, BL], F32, tag="hprz")
            hpn = phpn.tile([P, 2, BL], F32, tag="hpn")
            ssl = slice(BL * s, BL * (s + 1))
            # rz accumulation group: bias opener, 8 Whh mms, xp fold closer
            nc.tensor.matmul(hprz.rearrange("p a b -> p (a b)"),
                             biasmat[:], sel[:, 0:4 * BL],
                             start=True, stop=False)
            for k in range(2):
                for m in range(4):
                    wsl = slice(G * k + 128 * m, G * k + 128 * (m + 1))
                    last = not fold and (k == 1 and m == 3)
                    nc.tensor.matmul(hprz[:, m, :], whh[:, wsl], h[cur][:, k, :],
                                     start=False, stop=last)
            if fold:
                nc.tensor.matmul(hprz[:], ident[:], xp_t[:, 0:4, ssl],
                                 start=False, stop=True)
            # n accumulation group
            nc.tensor.matmul(hpn.rearrange("p a b -> p (a b)"),
                             biasmat[:], sel[:, 4 * BL:6 * BL],
                             start=True, stop=False)
            for k in range(2):
                for mi, m in enumerate((4, 5)):
                    wsl = slice(G * k + 128 * m, G * k + 128 * (m + 1))
                    nc.tensor.matmul(hpn[:, mi, :], whh[:, wsl], h[cur][:, k, :],
                                     start=False, stop=(k == 1 and mi == 1))

            rz = gsb.tile([P, 4, BL], F32, tag="rz")
            t1 = gsb.tile([P, 2, BL], F32, tag="t1")
            sn = gsb.tile([P, 2, BL], F32, tag="sn")
            zh = gsb.tile([P, 2, BL], F32, tag="zh")
            nt = gsb.tile([P, 2, BL], F32, tag="nt")
            t3m = gsb.tile([P, 2, BL], F32, tag="t3m")
            if fold:
                nc.scalar.activation(rz[:], hprz[:], AF.Sigmoid)
            else:
                scr = gsb.tile([P, 4, BL], F32, tag="scr")
                nc.vector.tensor_add(scr[:], xp_t[:, 0:4, ssl], hprz[:])
                nc.scalar.activation(rz[:], scr[:], AF.Sigmoid)
            nc.vector.tensor_mul(t1[:], rz[:, 0:2, :], hpn[:])
            nc.vector.tensor_add(sn[:], t1[:], xp_t[:, 4:6, ssl])
            # off-critical-path z*h (runs on DVE while ACT does tanh)
            nc.vector.tensor_mul(zh[:], rz[:, 2:4, :], h[cur][:])
            nc.scalar.activation(nt[:], sn[:], AF.Tanh)
            if stt:
                # h' = z*h - (z-1)*n = z*h + (1-z)*n
                nc.vector.scalar_tensor_tensor(
                    t3m[:], rz[:, 2:4, :], 1.0, nt[:],
                    op0=mybir.AluOpType.subtract, op1=mybir.AluOpType.mult)
                nc.vector.tensor_sub(h[nxt][:], zh[:], t3m[:])
            else:
                zc = gsb.tile([P, 2, BL], F32, tag="zc")
                nc.scalar.activation(zc[:], rz[:, 2:4, :], AF.Copy,
                                     scale=-1.0, bias=1.0)
                nc.vector.tensor_mul(t3m[:], nt[:], zc[:])
                nc.vector.tensor_add(h[nxt][:], t3m[:], zh[:])

        def emit_half(ci, parity):
            """Recurrence for chunk (2*ci+parity) reading xp2[parity];
            interleaved phase-1 for the next chunk into xp2[1-parity];
            prefetch DMA for chunk (2*ci+parity+2) into xin2[parity]."""
            cols = CH * BL
            if not nodma:
                nc.sync.dma_start(
                    xin2[parity][:],
                    d_xin.ap()[:, bass.ds(ci * (2 * cols) + (parity + 2) * cols, cols)])
            for s in range(CH):
                emit_step(s, xp2[parity])
                if s < n_interleave:
                    emit_p1_item(s, xin2[1 - parity], xp2[1 - parity])
                elif n_interleave == 0 and s == 0:
                    for it in range(12):
                        emit_p1_item(it, xin2[1 - parity], xp2[1 - parity])

        def emit_all():
            for t_ in (h[0],):
                nc.gpsimd.memset(t_[:], 0)
            # prologue: xin2[j] holds chunks of parity j throughout.
            nc.sync.dma_start(xin2[0][:], d_xin.ap()[:, 0:CH * BL])
            for it in range(12):
                emit_p1_item(it, xin2[0], xp2[0])
            # stage chunk 1 (consumed by half parity=0's interleaved phase-1)
            nc.sync.dma_start(xin2[1][:], d_xin.ap()[:, CH * BL:2 * CH * BL])
            with tc.For_i(
                0, halfiters,
                hint_engines=(mybir.EngineType.PE, mybir.EngineType.DVE),
            ) as ci:
                emit_half(ci, 0)
                emit_half(ci, 1)

        if reps > 1:
            with tc.For_i(0, reps, name="rep"):
                emit_all()
        else:
            emit_all()

        nc.sync.dma_start(d_out.ap()[:], h[0].rearrange("p a b -> p (a b)"))

    nc.compile()
    return nc


def _pack_v2(input, W_ih, W_hh, b_ih, b_hh, t_steps: int):
    input = np.asarray(input, np.float32)
    W_ih = np.asarray(W_ih, np.float32)
    W_hh = np.asarray(W_hh, np.float32)
    b_ih = np.asarray(b_ih, np.float32)
    b_hh = np.asarray(b_hh, np.float32)

    whhT = np.ascontiguousarray(W_hh.T)              # [H, G]
    whh = whhT.reshape(2, P, G).transpose(1, 0, 2).reshape(P, 2 * G)
    whh = np.ascontiguousarray(whh).astype(bf16)
    wih = np.ascontiguousarray(W_ih.T).astype(bf16)  # [128, 768]

    bias_full = b_hh.copy()
    bias_full[: 2 * H] += b_ih[: 2 * H]
    bmat32 = np.zeros((P, P), np.float32)
    bvec = bias_full.reshape(NBLK, P)
    bhi = bvec.astype(bf16).astype(np.float32)
    bmat32[0:NBLK, :] = bhi
    bmat32[NBLK: 2 * NBLK, :] = bvec - bhi
    biasmat = bmat32.astype(bf16)
    selmat = np.zeros((P, NBLK * BL), np.float32)
    for m in range(NBLK):
        selmat[m, BL * m: BL * (m + 1)] = 1.0
        selmat[m + NBLK, BL * m: BL * (m + 1)] = 1.0
    sel = selmat.astype(bf16)
    bihn = np.ascontiguousarray(b_ih[2 * H:].reshape(2, P).T)  # [128, 2]
    ident = np.eye(P, dtype=np.float32).astype(bf16)

    whhn = np.ascontiguousarray(-whh.astype(np.float32)).astype(bf16)
    shared = dict(whh=whh, whhn=whhn, wih=wih, biasmat=biasmat, sel=sel,
                  bihn=bihn, ident=ident)
    pad = 2 * CH * BL
    in_maps = []
    for c in range(NCORES):
        xs = input[c * BL: (c + 1) * BL, :t_steps, :]     # [16, t, 128]
        xt = np.ascontiguousarray(xs.transpose(2, 1, 0))  # [128, t, 16]
        xt = xt.reshape(P, t_steps * BL).astype(bf16)
        xin = np.zeros((P, t_steps * BL + pad), bf16)
        xin[:, :t_steps * BL] = xt
        m = dict(shared)
        m["xin"] = xin
        in_maps.append(m)
    return in_maps


def _unpack_v2(results):
    out = np.empty((B, H), np.float32)
    for c in range(NCORES):
        o = results[c]["hout"].astype(np.float32).reshape(P, 2, BL)
        out[c * BL: (c + 1) * BL, :] = o.transpose(2, 1, 0).reshape(BL, H)
    return out


def _build_v4(t_steps: int, reps: int = 1, n_interleave: int = 12,
              nodma: bool = False):
    """v4 = v2 + (F) r|z PSUM groups split so sigmoid_r starts sooner, and
    (G) split-feed: next-step matmuls consume bf16 z*h and (z-1)*n streams
    (negated weight copy for the subtraction), so the final h subtract is
    off the critical recurrence cycle.

    Critical cycle per step: STT -> 4 r-side t3m matmuls + stop -> psum
    drain -> ACT sigmoid_r -> DVE mul/add -> ACT tanh -> STT.
    """
    nchunks = t_steps // CH
    assert nchunks % 2 == 0
    halfiters = nchunks // 2
    nc = bacc.Bacc(
        "TRN2", target_bir_lowering=False, debug=False, num_devices=NCORES
    )

    d_xin = nc.dram_tensor("xin", [P, (t_steps + 2 * CH) * BL], BF16,
                           kind="ExternalInput")
    d_whh = nc.dram_tensor("whh", [P, 2 * G], BF16, kind="ExternalInput")
    d_whhn = nc.dram_tensor("whhn", [P, 2 * G], BF16, kind="ExternalInput")
    d_wih = nc.dram_tensor("wih", [P, G], BF16, kind="ExternalInput")
    d_biasmat = nc.dram_tensor("biasmat", [P, P], BF16, kind="ExternalInput")
    d_sel = nc.dram_tensor("sel", [P, NBLK * BL], BF16, kind="ExternalInput")
    d_bihn = nc.dram_tensor("bihn", [P, 2], F32, kind="ExternalInput")
    d_ident = nc.dram_tensor("ident", [P, P], BF16, kind="ExternalInput")
    d_out = nc.dram_tensor("hout", [P, 2 * BL], BF16, kind="ExternalOutput")

    with tile.TileContext(nc) as tc, ExitStack() as ctx:
        cpool = ctx.enter_context(tc.tile_pool(name="const", bufs=1))
        state = ctx.enter_context(tc.tile_pool(name="state", bufs=1))
        gsb = ctx.enter_context(tc.tile_pool(name="gates", bufs=2))
        phr_p = ctx.enter_context(tc.tile_pool(name="phr", bufs=2, space="PSUM"))
        phz_p = ctx.enter_context(tc.tile_pool(name="phz", bufs=2, space="PSUM"))
        phpn = ctx.enter_context(tc.tile_pool(name="phpn", bufs=2, space="PSUM"))
        px = ctx.enter_context(tc.tile_pool(name="px", bufs=2, space="PSUM"))

        whh = cpool.tile([P, 2 * G], BF16, tag="whh")
        whhn = cpool.tile([P, 2 * G], BF16, tag="whhn")
        wih = cpool.tile([P, G], BF16, tag="wih")
        biasmat = cpool.tile([P, P], BF16, tag="biasmat")
        sel = cpool.tile([P, NBLK * BL], BF16, tag="sel")
        bihn = cpool.tile([P, 2], F32, tag="bihn")
        ident = cpool.tile([P, P], BF16, tag="ident")
        for dst, src in ((whh, d_whh), (whhn, d_whhn), (wih, d_wih),
                         (biasmat, d_biasmat), (sel, d_sel), (bihn, d_bihn),
                         (ident, d_ident)):
            nc.sync.dma_start(dst[:], src.ap()[:])

        h = [state.tile([P, 2, BL], BF16, name=f"h{j}", tag=f"h{j}")
             for j in range(2)]
        zh2 = [state.tile([P, 2, BL], BF16, name=f"zh{j}", tag=f"zh{j}")
               for j in range(2)]
        t3m2 = [state.tile([P, 2, BL], BF16, name=f"t3m{j}", tag=f"t3m{j}")
                for j in range(2)]
        xp2 = [state.tile([P, NBLK, CH * BL], BF16, name=f"xp{j}", tag=f"xp{j}")
               for j in range(2)]
        xin2 = [state.tile([P, CH * BL], BF16, name=f"xin{j}", tag=f"xin{j}")
                for j in range(2)]

        def emit_p1_item(item, xin_t, xp_t):
            m, hf = item // 2, item % 2
            pxm = px.tile([P, 512], F32, tag="pxm")
            wsl = slice(128 * m, 128 * (m + 1))
            xsl = slice(512 * hf, 512 * (hf + 1))
            nc.tensor.matmul(pxm[:], wih[:, wsl], xin_t[:, xsl],
                             start=True, stop=True)
            if m < 4:
                nc.vector.tensor_copy(xp_t[:, m, xsl], pxm[:])
            else:
                nc.scalar.activation(xp_t[:, m, xsl], pxm[:], AF.Identity,
                                     bias=bihn[:, m - 4: m - 3])

        def emit_step(s, xp_t):
            cur, nxt = s % 2, (s + 1) % 2
            phr = phr_p.tile([P, 2, BL], F32, tag="phr")
            phz = phz_p.tile([P, 2, BL], F32, tag="phz")
            hpn = phpn.tile([P, 2, BL], F32, tag="hpn")
            ssl = slice(BL * s, BL * (s + 1))

            def hgroup(tgt, ms, selsl, foldsl):
                # opener: bias; then zh mms + xp fold (available early);
                # t3m mms LAST so the group closes right after STT.
                nc.tensor.matmul(tgt.rearrange("p a b -> p (a b)"),
                                 biasmat[:], sel[:, selsl],
                                 start=True, stop=False)
                for k in range(2):
                    for mi, m in enumerate(ms):
                        wsl = slice(G * k + 128 * m, G * k + 128 * (m + 1))
                        nc.tensor.matmul(tgt[:, mi, :], whh[:, wsl],
                                         zh2[cur][:, k, :],
                                         start=False, stop=False)
                if foldsl is not None:
                    nc.tensor.matmul(tgt[:], ident[:], xp_t[:, foldsl, ssl],
                                     start=False, stop=False)
                for k in range(2):
                    for mi, m in enumerate(ms):
                        wsl = slice(G * k + 128 * m, G * k + 128 * (m + 1))
                        nc.tensor.matmul(tgt[:, mi, :], whhn[:, wsl],
                                         t3m2[cur][:, k, :],
                                         start=False,
                                         stop=(k == 1 and mi == len(ms) - 1))

            hgroup(phr, (0, 1), slice(0, 2 * BL), slice(0, 2))
            hgroup(phz, (2, 3), slice(2 * BL, 4 * BL), slice(2, 4))
            hgroup(hpn, (4, 5), slice(4 * BL, 6 * BL), None)

            rt = gsb.tile([P, 2, BL], F32, tag="rt")
            zt = gsb.tile([P, 2, BL], F32, tag="zt")
            t1 = gsb.tile([P, 2, BL], F32, tag="t1")
            sn = gsb.tile([P, 2, BL], F32, tag="sn")
            nt = gsb.tile([P, 2, BL], F32, tag="nt")
            nc.scalar.activation(rt[:], phr[:], AF.Sigmoid)
            nc.scalar.activation(zt[:], phz[:], AF.Sigmoid)
            nc.vector.tensor_mul(t1[:], rt[:], hpn[:])
            nc.vector.tensor_add(sn[:], t1[:], xp_t[:, 4:6, ssl])
            # z*h for the NEXT step's feed (off the critical cycle)
            nc.vector.tensor_mul(zh2[nxt][:], zt[:], h[cur][:])
            nc.scalar.activation(nt[:], sn[:], AF.Tanh)
            nc.vector.scalar_tensor_tensor(
                t3m2[nxt][:], zt[:], 1.0, nt[:],
                op0=mybir.AluOpType.subtract, op1=mybir.AluOpType.mult)
            # materialize h for the next z*h and the final output (off-cycle)
            nc.vector.tensor_sub(h[nxt][:], zh2[nxt][:], t3m2[nxt][:])

        def emit_half(ci, parity):
            cols = CH * BL
            if not nodma:
                nc.sync.dma_start(
                    xin2[parity][:],
                    d_xin.ap()[:, bass.ds(ci * (2 * cols) + (parity + 2) * cols,
                                          cols)])
            for s in range(CH):
                emit_step(s, xp2[parity])
                if s < n_interleave:
                    emit_p1_item(s, xin2[1 - parity], xp2[1 - parity])

        def emit_all():
            for t_ in (h[0], zh2[0], t3m2[0]):
                nc.gpsimd.memset(t_[:], 0)
            nc.sync.dma_start(xin2[0][:], d_xin.ap()[:, 0:CH * BL])
            for it in range(12):
                emit_p1_item(it, xin2[0], xp2[0])
            nc.sync.dma_start(xin2[1][:], d_xin.ap()[:, CH * BL:2 * CH * BL])
            with tc.For_i(
                0, halfiters,
                hint_engines=(mybir.EngineType.PE, mybir.EngineType.DVE),
            ) as ci:
                emit_half(ci, 0)
                emit_half(ci, 1)

        if reps > 1:
            with tc.For_i(0, reps, name="rep"):
                emit_all()
        else:
            emit_all()

        nc.sync.dma_start(d_out.ap()[:], h[0].rearrange("p a b -> p (a b)"))

    nc.compile()
    return nc


# ---------------------------------------------------------------------------
# v3: fully-DVE gate chain via custom polynomial activation micro-ops.
# sigma_z stays exact on ACT (parallel, off critical path); z*h on GPSIMD.
# tanh ~ clamp(x*P6(x^2)) (3 DVE insts), r folded into t1 via
# clamp(x*Q4+0.5,0,1)*hpn (3 DVE insts incl. the multiply).
# ---------------------------------------------------------------------------

# minimax coefficients: tanh ~ clamp(x*P6(x^2)) fit on [0,3.6] (global err
# 2.2e-3), sigma ~ clamp(0.5 + x*Q4(x^2), 0, 1) fit on [0,6.0] (err 2.7e-3).
_CT = [0.98913864512, -0.28780500403, 0.073307507501, -0.012014940477,
       0.0011548376344, -5.8764841675e-05, 1.21686344e-06]
_CS = [0.24396367529, -0.015843419093, 0.00076713848496, -1.9324037226e-05,
       1.8881984989e-07]

_POLY_OPS: dict = {}


def _register_poly_ops():
    """Define + register the custom DVE ops at runtime (repo is read-only;
    the documented flow is appending to dve_ops.OPS)."""
    if _POLY_OPS:
        return _POLY_OPS
    import numpy as _np
    import concourse.dve_ops as _dops
    from concourse.dve_spec import (
        Spec, Src0, Src1, C0, C1, C2, Zero, One, sq, minn, maxx, lower,
        _has_src1,
    )
    from concourse.dve_table_gen import dve_ver_for
    from concourse.dve_uop import DveOpSpec

    t = sq(Src0)
    bodies = {
        # w = ((c_a*t + c_b)*t + c_c)*t
        "ANT_POLY1": (
            ((C0 * t + C1) * t + C2) * t,
            lambda in0, in1, s0, s1, imm2:
                (((s0 * (in0 * in0) + s1) * (in0 * in0) + imm2)
                 * (in0 * in0)).astype(_np.float32)),
        # out = ((w + c_a)*t + c_b)*t
        "ANT_POLY2T": (
            ((Src1 + C0) * t + C1) * t,
            lambda in0, in1, s0, s1, imm2:
                (((in1 + s0) * (in0 * in0) + s1)
                 * (in0 * in0)).astype(_np.float32)),
        # out = ((w + c_a)*t + c_b)*x  (sigma pre-clamp)
        "ANT_POLY2X": (
            ((Src1 + C0) * t + C1) * Src0,
            lambda in0, in1, s0, s1, imm2:
                (((in1 + s0) * (in0 * in0) + s1) * in0).astype(_np.float32)),
        # out = clamp(y + c0, 0, 1) * in1  (finish sigma, multiply by hpn)
        "ANT_SIGFIN_MUL": (
            maxx(minn(Src0 + C0, One), Zero) * Src1,
            lambda in0, in1, s0, s1, imm2:
                (_np.clip(in0 + s0, 0.0, 1.0) * in1).astype(_np.float32)),
        # out = clamp(((w + c_a)*t + c_b)*x, -1, 1)  (finish tanh)
        "ANT_POLY3C": (
            maxx(minn(((Src1 + C0) * t + C1) * Src0, One), Zero - One),
            lambda in0, in1, s0, s1, imm2:
                _np.clip(((in1 + s0) * (in0 * in0) + s1) * in0,
                         -1.0, 1.0).astype(_np.float32)),
    }
    base = max(_dops._SUB_OPCODE_FOR_NAME.values()) + 1
    for i, (name, (body, ref)) in enumerate(bodies.items()):
        if name in _dops._SUB_OPCODE_FOR_NAME:
            _POLY_OPS[name] = next(o for o in _dops.OPS if o.name == name)
            continue
        spec = Spec(body=body, reference=ref)
        shas = {}
        for ver in ("v3", "v4"):
            u = lower(spec, ver=ver)
            shas[ver] = DveOpSpec(name=name, uops=u,
                                  rd1_en=_has_src1(spec)).sha(ver)
        op = _dops.DveOp(name, spec, subdim=False, uops_sha=shas)
        _dops._SUB_OPCODE_FOR_NAME[name] = base + i
        _dops.OPS.append(op)
        _dops.CUSTOM_DVE_SPECS[name] = spec
        _POLY_OPS[name] = op
    return _POLY_OPS


def _build_v3(t_steps: int, reps: int = 1, n_interleave: int = 12):
    ops = _register_poly_ops()
    P1, P2T, P2X, SFM, P3C = (ops["ANT_POLY1"], ops["ANT_POLY2T"],
                              ops["ANT_POLY2X"], ops["ANT_SIGFIN_MUL"],
                              ops["ANT_POLY3C"])
    ct, cs = _CT, _CS
    nchunks = t_steps // CH
    assert nchunks % 2 == 0
    halfiters = nchunks // 2
    nc = bacc.Bacc(
        "TRN2", target_bir_lowering=False, debug=False, num_devices=NCORES
    )

    d_xin = nc.dram_tensor("xin", [P, (t_steps + 2 * CH) * BL], BF16,
                           kind="ExternalInput")
    d_whh = nc.dram_tensor("whh", [P, 2 * G], BF16, kind="ExternalInput")
    d_wih = nc.dram_tensor("wih", [P, G], BF16, kind="ExternalInput")
    d_biasmat = nc.dram_tensor("biasmat", [P, P], BF16, kind="ExternalInput")
    d_sel = nc.dram_tensor("sel", [P, NBLK * BL], BF16, kind="ExternalInput")
    d_bihn = nc.dram_tensor("bihn", [P, 2], F32, kind="ExternalInput")
    d_ident = nc.dram_tensor("ident", [P, P], BF16, kind="ExternalInput")
    d_out = nc.dram_tensor("hout", [P, 2 * BL], BF16, kind="ExternalOutput")

    with tile.TileContext(nc) as tc, ExitStack() as ctx:
        cpool = ctx.enter_context(tc.tile_pool(name="const", bufs=1))
        state = ctx.enter_context(tc.tile_pool(name="state", bufs=1))
        gsb = ctx.enter_context(tc.tile_pool(name="gates", bufs=2))
        php = ctx.enter_context(tc.tile_pool(name="php", bufs=2, space="PSUM"))
        phpn = ctx.enter_context(tc.tile_pool(name="phpn", bufs=2, space="PSUM"))
        px = ctx.enter_context(tc.tile_pool(name="px", bufs=2, space="PSUM"))

        whh = cpool.tile([P, 2 * G], BF16, tag="whh")
        wih = cpool.tile([P, G], BF16, tag="wih")
        biasmat = cpool.tile([P, P], BF16, tag="biasmat")
        sel = cpool.tile([P, NBLK * BL], BF16, tag="sel")
        bihn = cpool.tile([P, 2], F32, tag="bihn")
        ident = cpool.tile([P, P], BF16, tag="ident")
        for dst, src in ((whh, d_whh), (wih, d_wih), (biasmat, d_biasmat),
                         (sel, d_sel), (bihn, d_bihn), (ident, d_ident)):
            nc.sync.dma_start(dst[:], src.ap()[:])

        h = [state.tile([P, 2, BL], BF16, name=f"h{j}", tag=f"h{j}")
             for j in range(2)]
        xp2 = [state.tile([P, NBLK, CH * BL], BF16, name=f"xp{j}", tag=f"xp{j}")
               for j in range(2)]
        xin2 = [state.tile([P, CH * BL], BF16, name=f"xin{j}", tag=f"xin{j}")
                for j in range(2)]

        def emit_p1_item(item, xin_t, xp_t):
            m, hf = item // 2, item % 2
            pxm = px.tile([P, 512], F32, tag="pxm")
            wsl = slice(128 * m, 128 * (m + 1))
            xsl = slice(512 * hf, 512 * (hf + 1))
            nc.tensor.matmul(pxm[:], wih[:, wsl], xin_t[:, xsl],
                             start=True, stop=True)
            if m < 4:
                nc.vector.tensor_copy(xp_t[:, m, xsl], pxm[:])
            else:
                nc.scalar.activation(xp_t[:, m, xsl], pxm[:], AF.Identity,
                                     bias=bihn[:, m - 4: m - 3])

        def emit_step(s, xp_t):
            cur, nxt = s % 2, (s + 1) % 2
            hprz = php.tile([P, 4, BL], F32, tag="hprz")
            hpn = phpn.tile([P, 2, BL], F32, tag="hpn")
            ssl = slice(BL * s, BL * (s + 1))
            nc.tensor.matmul(hprz.rearrange("p a b -> p (a b)"),
                             biasmat[:], sel[:, 0:4 * BL],
                             start=True, stop=False)
            for k in range(2):
                for m in range(4):
                    wsl = slice(G * k + 128 * m, G * k + 128 * (m + 1))
                    nc.tensor.matmul(hprz[:, m, :], whh[:, wsl], h[cur][:, k, :],
                                     start=False, stop=False)
            nc.tensor.matmul(hprz[:], ident[:], xp_t[:, 0:4, ssl],
                             start=False, stop=True)
            nc.tensor.matmul(hpn.rearrange("p a b -> p (a b)"),
                             biasmat[:], sel[:, 4 * BL:6 * BL],
                             start=True, stop=False)
            for k in range(2):
                for mi, m in enumerate((4, 5)):
                    wsl = slice(G * k + 128 * m, G * k + 128 * (m + 1))
                    nc.tensor.matmul(hpn[:, mi, :], whh[:, wsl], h[cur][:, k, :],
                                     start=False, stop=(k == 1 and mi == 1))

            hr = hprz[:, 0:2, :].rearrange("p a b -> p (a b)")   # [P,32] PSUM
            hz = hprz[:, 2:4, :].rearrange("p a b -> p (a b)")
            hpn_f = hpn.rearrange("p a b -> p (a b)")
            F = 2 * BL
            w1 = gsb.tile([P, F], F32, tag="w1")
            y2 = gsb.tile([P, F], F32, tag="y2")
            t1 = gsb.tile([P, F], F32, tag="t1")
            sn = gsb.tile([P, F], F32, tag="sn")
            tw1 = gsb.tile([P, F], F32, tag="tw1")
            tw2 = gsb.tile([P, F], F32, tag="tw2")
            nt = gsb.tile([P, F], F32, tag="nt")
            zt = gsb.tile([P, F], F32, tag="zt")
            zh = gsb.tile([P, F], F32, tag="zh")
            t3m = gsb.tile([P, F], F32, tag="t3m")
            # r/n chain, all DVE. Emitted BEFORE the ACT/Pool z-path: tile
            # readers of the same psum tile are chained in emission order,
            # so the critical DVE chain must read hprz first.
            nc.vector._custom_dve(P1, out=w1[:], in0=hr,
                                  s0=cs[4], s1=cs[3], imm2=cs[2])
            nc.vector._custom_dve(P2X, out=y2[:], in0=hr, in1=w1[:],
                                  s0=cs[1], s1=cs[0])
            # z path: exact sigmoid on ACT, z*h on GPSIMD — both parallel to
            # the rest of the DVE chain.
            nc.scalar.activation(zt[:], hz, AF.Sigmoid)
            nc.gpsimd.tensor_mul(zh[:], zt[:],
                                 h[cur].rearrange("p a b -> p (a b)"))
            nc.vector._custom_dve(SFM, out=t1[:], in0=y2[:], in1=hpn_f,
                                  s0=0.5)
            nc.vector.tensor_add(sn[:], t1[:], xp_t[:, 4:6, ssl])
            nc.vector._custom_dve(P1, out=tw1[:], in0=sn[:],
                                  s0=ct[6], s1=ct[5], imm2=ct[4])
            nc.vector._custom_dve(P2T, out=tw2[:], in0=sn[:], in1=tw1[:],
                                  s0=ct[3], s1=ct[2])
            nc.vector._custom_dve(P3C, out=nt[:], in0=sn[:], in1=tw2[:],
                                  s0=ct[1], s1=ct[0])
            nc.vector.scalar_tensor_tensor(
                t3m[:], zt[:], 1.0, nt[:],
                op0=mybir.AluOpType.subtract, op1=mybir.AluOpType.mult)
            nc.vector.tensor_sub(h[nxt].rearrange("p a b -> p (a b)"),
                                 zh[:], t3m[:])

        def emit_half(ci, parity):
            cols = CH * BL
            nc.sync.dma_start(
                xin2[parity][:],
                d_xin.ap()[:, bass.ds(ci * (2 * cols) + (parity + 2) * cols, cols)])
            for s in range(CH):
                emit_step(s, xp2[parity])
                if s < n_interleave:
                    emit_p1_item(s, xin2[1 - parity], xp2[1 - parity])

        def emit_all():
            nc.gpsimd.memset(h[0][:], 0)
            nc.sync.dma_start(xin2[0][:], d_xin.ap()[:, 0:CH * BL])
            for it in range(12):
                emit_p1_item(it, xin2[0], xp2[0])
            nc.sync.dma_start(xin2[1][:], d_xin.ap()[:, CH * BL:2 * CH * BL])
            with tc.For_i(
                0, halfiters,
                hint_engines=(mybir.EngineType.PE, mybir.EngineType.DVE),
            ) as ci:
                emit_half(ci, 0)
                emit_half(ci, 1)

        if reps > 1:
            with tc.For_i(0, reps, name="rep"):
                emit_all()
        else:
            emit_all()

        nc.sync.dma_start(d_out.ap()[:], h[0].rearrange("p a b -> p (a b)"))

    nc.compile()
    return nc


_PROGRAM_CACHE: dict = {}


def _get_program(t_steps: int, reps: int = 1, ver: int = 2):
    key = (t_steps, reps, ver)
    if key not in _PROGRAM_CACHE:
        builder = {1: _build_program, 2: _build_v2, 3: _build_v3,
                   4: _build_v4}[ver]
        _PROGRAM_CACHE[key] = builder(t_steps, reps)
    return _PROGRAM_CACHE[key]


def _pack_inputs(input, W_ih, W_hh, b_ih, b_hh, t_steps: int):
    """Host-side packing. Returns per-core in_maps."""
    input = np.asarray(input, np.float32)
    W_ih = np.asarray(W_ih, np.float32)
    W_hh = np.asarray(W_hh, np.float32)
    b_ih = np.asarray(b_ih, np.float32)
    b_hh = np.asarray(b_hh, np.float32)

    # weights, feature-major packed (shared by all cores)
    whhT = np.ascontiguousarray(W_hh.T)              # [H, G]
    whh = whhT.reshape(2, P, G).transpose(1, 0, 2).reshape(P, 2 * G)
    whh_hi, whh_lo = _split_hi_lo(np.ascontiguousarray(whh))
    wihT = np.ascontiguousarray(W_ih.T)              # [X, G] = [128, 768]
    wih_hi, wih_lo = _split_hi_lo(wihT)

    # bias matrix: rows 0..5 hi parts, rows 6..11 lo parts; selector picks both
    bias_full = b_hh.copy()
    bias_full[: 2 * H] += b_ih[: 2 * H]              # r,z: b_ih + b_hh; n: b_hh
    bmat32 = np.zeros((P, P), np.float32)
    bvec = bias_full.reshape(NBLK, P)
    bhi = bvec.astype(bf16).astype(np.float32)
    blo = bvec - bhi
    bmat32[0:NBLK, :] = bhi
    bmat32[NBLK: 2 * NBLK, :] = blo
    biasmat = bmat32.astype(bf16)
    selmat = np.zeros((P, NBLK * BL), np.float32)
    for m in range(NBLK):
        selmat[m, BL * m: BL * (m + 1)] = 1.0
        selmat[m + NBLK, BL * m: BL * (m + 1)] = 1.0
    sel = selmat.astype(bf16)
    bihn = np.ascontiguousarray(b_ih[2 * H:].reshape(2, P).T)  # [128, 2]

    shared = dict(
        whh_hi=whh_hi, whh_lo=whh_lo, wih_hi=wih_hi, wih_lo=wih_lo,
        biasmat=biasmat, sel=sel, bihn=bihn,
        whh_f=np.ascontiguousarray(whh), biasmat_f=bmat32, sel_f=selmat,
    )
    in_maps = []
    for c in range(NCORES):
        xs = input[c * BL: (c + 1) * BL, :t_steps, :]     # [16, t, 128]
        xt = np.ascontiguousarray(xs.transpose(2, 1, 0))  # [128, t, 16]
        xt = xt.reshape(P, t_steps * BL)
        xh, xl = _split_hi_lo(xt)
        m = dict(shared)
        m["xin_hi"] = xh
        m["xin_lo"] = xl
        in_maps.append(m)
    return in_maps


def _unpack_output(results):
    out = np.empty((B, H), np.float32)
    for c in range(NCORES):
        o = results[c]["hout"].reshape(P, 2, BL)           # [p, k, b]
        out[c * BL: (c + 1) * BL, :] = o.transpose(2, 1, 0).reshape(BL, H)
    return out


VER = 2


def run(input, W_ih, W_hh, b_ih, b_hh, t_steps: int = T, trace: bool = False,
        ver: int = None):
    ver = VER if ver is None else ver
    nc = _get_program(t_steps, ver=ver)
    pack = _pack_inputs if ver == 1 else _pack_v2
    unpack = _unpack_output if ver == 1 else _unpack_v2
    in_maps = pack(input, W_ih, W_hh, b_ih, b_hh, t_steps)
    res = run_bass_kernel_spmd(
        nc, in_maps, core_ids=list(range(NCORES)), trace=trace
    )
    return unpack(res.results), res


def kernel(input, W_ih, W_hh, b_ih, b_hh):
    out, _ = run(input, W_ih, W_hh, b_ih, b_hh)
    return out


def bench(input, W_ih, W_hh, b_ih, b_hh, reps_hi: int = 5, iters: int = 3,
          ver: int = None):
    """Estimate on-device time: wall(R=reps_hi) - wall(R=1) over cached
    executables, divided by (reps_hi - 1). Returns ns."""
    import time as _time

    ver = VER if ver is None else ver
    pack = _pack_v2 if ver == 2 else _pack_inputs
    in_maps = pack(input, W_ih, W_hh, b_ih, b_hh, T)
    nc1 = _get_program(T, 1, ver=ver)
    ncR = _get_program(T, reps_hi, ver=ver)

    def timed(nc):
        best = float("inf")
        for _ in range(iters):
            t0 = _time.perf_counter()
            run_bass_kernel_spmd(nc, in_maps, core_ids=list(range(NCORES)))
            best = min(best, _time.perf_counter() - t0)
        return best

    # warm both executables (compile cache)
    run_bass_kernel_spmd(nc1, in_maps, core_ids=list(range(NCORES)))
    run_bass_kernel_spmd(ncR, in_maps, core_ids=list(range(NCORES)))
    t1 = timed(nc1)
    tR = timed(ncR)
    ns = (tR - t1) / (reps_hi - 1) * 1e9
    print(f"wall R=1: {t1*1e3:.1f} ms   wall R={reps_hi}: {tR*1e3:.1f} ms")
    return ns

